# revision 1
# baseline (speedup 1.0000x reference)
"""Trainium2 Bass kernel for nn_EvolutionBlock (moe_routing).

Strategy: data-parallel over the 8192 tokens across 8 NeuronCores
(1024 tokens/core + 3-token halo for the causal conv). Weights are
replicated per core and pre-packed on the host into the exact
[128, cols] SBUF layouts so every DMA is a contiguous slab.

On-chip everything is feature-major ([feature, token]) so matmuls are
out[f_chunk, tok] = lhsT.T @ rhs with lhsT = weight tile [din, dout]
and rhs = activation [din, tok]. Router/top-2 runs token-major in fp32
(selection must match the fp32 reference argmax), gets transposed via
the PE, and the per-token weights are broadcast across partitions with
K=1 ones-matmuls. Branch combine weights are folded into the fc2
inputs so each branch's final matmul accumulates the pre-scaled
contribution straight into PSUM; all branch biases collapse into one
[10, D] bias matmul against the router-weight rows.
"""

import numpy as np
import ml_dtypes

import concourse.bass as bass
import concourse.tile as tile
from concourse import bacc, mybir
from concourse import bass_utils

F32 = mybir.dt.float32
BF16 = mybir.dt.bfloat16
AF = mybir.ActivationFunctionType
ALU = mybir.AluOpType
AX = mybir.AxisListType
BF = ml_dtypes.bfloat16

# Problem constants
B, T, D = 4, 2048, 1024
HD = 4096          # dense hidden (fc1 out = 2*HD)
S, KC_ = 1024, 4   # ssm state, conv kernel
E, HE = 8, 512     # experts, expert hidden
NCORE = 8
TOKENS = B * T
TOK = TOKENS // NCORE   # tokens per core
HALO = 3
DC = D // 128           # 8 d-chunks


def _coltiles(n, w=512):
    out = []
    c = 0
    while c < n:
        out.append((c, min(w, n - c)))
        c += w
    return out




def build_program(ntok=TOK):
    """Build + compile the Bass program for `ntok` tokens per core.

    Phase order: hT -> routers -> conv -> ssm-out -> MoE -> dense.
    Pool lifetimes overlap across phase boundaries so the Tile
    scheduler can fill one phase's PE stalls with the next phase's
    matmuls; PSUM stays within 8 banks at every overlap point.
    """
    nt = ntok + HALO
    nc = bacc.Bacc("TRN2", target_bir_lowering=False, debug=False,
                   num_devices=NCORE)

    def din(name, shape, dt):
        return nc.dram_tensor(name, list(shape), dt, kind="ExternalInput").ap()

    xl_d = din("xl_s", [128, DC * ntok], BF16)
    xs_d = din("x_s", [128, DC * nt], BF16)
    wrmh_d = din("w_rmh", [128, DC * 11], BF16)
    wrml_d = din("w_rml", [128, DC * 11], BF16)
    rmb_d = din("rm_bias", [11, 1], F32)
    id11_d = din("ident11", [11, 11], F32)
    ident_d = din("ident", [128, 128], BF16)
    ones_d = din("ones1", [1, 128], BF16)
    wsin_d = din("w_sin", [128, 64 * 128], BF16)
    bsin_d = din("b_sin", [128, 8], F32)
    wconv_d = din("w_conv", [128, 8 * 32 * 128], BF16)
    bconv_d = din("b_conv", [128, 8], F32)
    wsout_d = din("w_sout", [128, 64 * 128], BF16)
    b10_d = din("b10", [10, 1024], BF16)
    we1_d = din("w_e1", [128, E * 64 * 128], BF16)
    be1a_d = din("b_e1a", [128, 32], F32)
    be1b_d = din("b_e1b", [128, 32], F32)
    we2_d = din("w_e2", [128, E * 32 * 128], BF16)
    wd1a_d = din("w_d1a", [128, 256 * 128], BF16)
    wd1b_d = din("w_d1b", [128, 256 * 128], BF16)
    bd1a_d = din("b_d1a", [128, 32], F32)
    bd1b_d = din("b_d1b", [128, 32], F32)
    wd2_d = din("w_d2", [128, 256 * 128], BF16)

    out_d = nc.dram_tensor("outT", [128, DC * ntok], F32,
                           kind="ExternalOutput").ap()

    cts = _coltiles(ntok)
    cts_h = _coltiles(nt)
    nchunk = ntok // 128

    with tile.TileContext(nc) as tc:
        live = []

        def P(name, bufs, space="SBUF", side="left"):
            p = tc.alloc_tile_pool(name=name, bufs=bufs, space=space,
                                   side=side)
            live.append(p)
            return p

        def rel(*ps):
            for p in ps:
                live.remove(p)
                p.release()

        constp = P("constp", 1)
        xp = P("xp", 1)

        # H-phase pools first so the first-needed DMAs issue first:
        # wsin slab 0, then the first column-half of each x block.
        cp = P("cp", 1, side="right")
        c_s = cp.tile([128, DC * ntok], BF16)
        hp = P("hp", 1)
        hw = P("hw", 1, side="right")
        hps = P("hps", 3, "PSUM")
        h_s = hp.tile([128, DC * nt], BF16)
        wsin = hw.tile([128, 64 * 128], BF16)
        x_s = xp.tile([128, DC * nt], BF16)
        nc.sync.dma_start(wsin[:, 0:1024], wsin_d[:, 0:1024])
        nc.sync.dma_start(x_s[:], xs_d[:])
        for mc in range(1, DC):
            nc.sync.dma_start(wsin[:, mc * 1024:(mc + 1) * 1024],
                              wsin_d[:, mc * 1024:(mc + 1) * 1024])
        ident = constp.tile([128, 128], BF16)
        nc.sync.dma_start(ident[:], ident_d[:])
        ones1 = constp.tile([1, 128], BF16)
        nc.sync.dma_start(ones1[:], ones_d[:])
        rm_bias = constp.tile([11, 1], F32)
        nc.sync.dma_start(rm_bias[:], rmb_d[:])
        ident11 = constp.tile([11, 11], F32)
        nc.sync.dma_start(ident11[:], id11_d[:])
        b_sin = constp.tile([128, 8], F32)
        nc.sync.dma_start(b_sin[:], bsin_d[:])
        b_conv = constp.tile([128, 8], F32)
        nc.sync.dma_start(b_conv[:], bconv_d[:])
        b10 = constp.tile([10, 1024], BF16)
        nc.sync.dma_start(b10[:], b10_d[:])
        b_e1a = constp.tile([128, 32], F32)
        nc.sync.dma_start(b_e1a[:], be1a_d[:])
        b_e1b = constp.tile([128, 32], F32)
        nc.sync.dma_start(b_e1b[:], be1b_d[:])
        b_d1a = constp.tile([128, 32], F32)
        nc.sync.dma_start(b_d1a[:], bd1a_d[:])
        b_d1b = constp.tile([128, 32], F32)
        nc.sync.dma_start(b_d1b[:], bd1b_d[:])
        rw10 = constp.tile([10, ntok], BF16)
        rwrows = [constp.tile([1, ntok], BF16, tag=f"rwrow{r}",
                              name=f"rwrow{r}") for r in range(10)]
        out_acc = constp.tile([128, DC * ntok], F32)

        def bcast_row(r, pool, pspool, tag):
            """[128, ntok] bf16 broadcast of rw10 row r (K=1 matmul)."""
            wbt = pool.tile([128, ntok], BF16, tag=tag, name=tag)
            for (c0, cw) in cts:
                pb = pspool.tile([128, 512], F32, tag="pb", name="pb",
                                 bufs=1)
                nc.tensor.matmul(pb[:, :cw], ones1[:],
                                 rwrows[r][:, c0:c0 + cw],
                                 start=True, stop=True)
                nc.scalar.copy(wbt[:, c0:c0 + cw], pb[:, :cw])
            return wbt

        # ================= Phase H: hT = sW_in @ x =================
        for mc in range(DC):
            for (c0, cw) in cts_h:
                ps = hps.tile([128, 512], F32, tag="hpsum", name="hpsum")
                for kc in range(DC):
                    nc.tensor.matmul(
                        ps[:, :cw],
                        wsin[:, (mc * 8 + kc) * 128:(mc * 8 + kc + 1) * 128],
                        x_s[:, kc * nt + c0:kc * nt + c0 + cw],
                        start=(kc == 0), stop=(kc == DC - 1))
                nc.scalar.activation(
                    h_s[:, mc * nt + c0:mc * nt + c0 + cw],
                    ps[:, :cw], AF.Identity, bias=b_sin[:, mc:mc + 1])

        # ================= Phase R: routers (stage-major) ==========
        rxp = P("rxp", 1, side="right")
        rp = P("rp", nchunk, side="right")
        rps = P("rps", 1, "PSUM", side="right")
        xl_s = rxp.tile([128, DC * ntok], BF16)
        nc.sync.dma_start(xl_s[:], xl_d[:])
        wrmh = rxp.tile([128, DC * 11], BF16)
        nc.sync.dma_start(wrmh[:], wrmh_d[:])
        wrml = rxp.tile([128, DC * 11], BF16)
        nc.sync.dma_start(wrml[:], wrml_d[:])

        rsbs, e3s, tm10s = [], [], []
        # stage 1: exact logits feature-major (3-term bf16 hi/lo),
        # then transpose [11,128]-chunks back to token-major
        lg = rxp.tile([11, ntok], F32, tag="lg", name="lg")
        for (c0, cw) in cts:
            ps = rps.tile([11, 512], F32, tag="ps", name="ps")
            nmm = 3 * DC
            im = 0
            for kc in range(DC):
                xh_c = x_s[:, kc * nt + HALO + c0:kc * nt + HALO + c0 + cw]
                xl_c = xl_s[:, kc * ntok + c0:kc * ntok + c0 + cw]
                wh_c = wrmh[:, kc * 11:(kc + 1) * 11]
                wl_c = wrml[:, kc * 11:(kc + 1) * 11]
                for (lhs_c, rhs_c) in ((wh_c, xh_c), (wl_c, xh_c),
                                       (wh_c, xl_c)):
                    nc.tensor.matmul(ps[:, :cw], lhs_c, rhs_c,
                                     start=(im == 0), stop=(im == nmm - 1))
                    im += 1
            nc.scalar.activation(lg[:, c0:c0 + cw], ps[:, :cw], AF.Identity,
                                 bias=rm_bias[:, 0:1])
        for tcn in range(nchunk):
            pst2 = rps.tile([128, 11], F32, tag="pst2", name="pst2")
            nc.tensor.transpose(pst2[:],
                                lg[:, tcn * 128:(tcn + 1) * 128], ident11[:])
            rsb = rp.tile([128, 11], F32, tag="rsb", name="rsb")
            nc.scalar.copy(rsb[:], pst2[:])
            e3 = rp.tile([128, 3], F32, tag="e3", name="e3")
            nc.scalar.activation(e3[:], rsb[:, 0:3], AF.Exp)
            rsbs.append(rsb)
            e3s.append(e3)
        # stage 2: top-2 + branch weights
        for tcn in range(nchunk):
            rsb, e3 = rsbs[tcn], e3s[tcn]
            s3 = rp.tile([128, 1], F32, tag="s3", name="s3")
            nc.vector.reduce_sum(s3[:], e3[:], axis=AX.X)
            r3 = rp.tile([128, 1], F32, tag="r3", name="r3")
            nc.vector.reciprocal(r3[:], s3[:])
            tm10 = rp.tile([128, 10], BF16, tag="tm10", name="tm10")
            nc.vector.tensor_scalar(out=tm10[:, 0:2], in0=e3[:, 0:2],
                                    scalar1=r3[:], scalar2=None, op0=ALU.mult)
            bw2 = rp.tile([128, 1], F32, tag="bw2", name="bw2")
            nc.vector.tensor_scalar(out=bw2[:], in0=e3[:, 2:3], scalar1=r3[:],
                                    scalar2=None, op0=ALU.mult)
            L = rsb[:, 3:11]
            m1 = rp.tile([128, 1], F32, tag="m1", name="m1")
            nc.vector.reduce_max(m1[:], L, axis=AX.X)
            mask1 = rp.tile([128, 8], F32, tag="mask1", name="mask1")
            nc.vector.tensor_scalar(out=mask1[:], in0=L, scalar1=m1[:],
                                    scalar2=None, op0=ALU.is_equal)
            L2 = rp.tile([128, 8], F32, tag="L2", name="L2")
            nc.vector.scalar_tensor_tensor(out=L2[:], in0=mask1[:],
                                           scalar=-1e9, in1=L,
                                           op0=ALU.mult, op1=ALU.add)
            m2 = rp.tile([128, 1], F32, tag="m2", name="m2")
            nc.vector.reduce_max(m2[:], L2[:], axis=AX.X)
            mask2 = rp.tile([128, 8], F32, tag="mask2", name="mask2")
            nc.vector.tensor_scalar(out=mask2[:], in0=L2[:], scalar1=m2[:],
                                    scalar2=None, op0=ALU.is_equal)
            dv = rp.tile([128, 1], F32, tag="dv", name="dv")
            nc.vector.tensor_sub(dv[:], m1[:], m2[:])
            w1 = rp.tile([128, 1], F32, tag="w1", name="w1")
            nc.scalar.activation(w1[:], dv[:], AF.Sigmoid)
            u1 = rp.tile([128, 1], F32, tag="u1", name="u1")
            nc.vector.tensor_mul(u1[:], w1[:], bw2[:])
            u2 = rp.tile([128, 1], F32, tag="u2", name="u2")
            nc.vector.tensor_sub(u2[:], bw2[:], u1[:])
            c2t = rp.tile([128, 8], F32, tag="c2t", name="c2t")
            nc.vector.tensor_scalar(out=c2t[:], in0=mask2[:], scalar1=u2[:],
                                    scalar2=None, op0=ALU.mult)
            nc.vector.scalar_tensor_tensor(out=tm10[:, 2:10], in0=mask1[:],
                                           scalar=u1[:], in1=c2t[:],
                                           op0=ALU.mult, op1=ALU.add)
            tm10s.append(tm10)
        rel(hps)
        # stage 3: transposes -> rw10 + per-row vectors
        for tcn in range(nchunk):
            tm10 = tm10s[tcn]
            pst = rps.tile([10, 128], BF16, tag="pst2", name="pst")
            nc.tensor.transpose(pst[:], tm10[:], ident[:])
            nc.scalar.copy(rw10[:, tcn * 128:(tcn + 1) * 128], pst[:])
            for r in range(10):
                pr = rps.tile([1, 128], BF16, tag="pr", name="pr", bufs=2)
                nc.tensor.transpose(pr[:], tm10[:, r:r + 1], ident[:])
                nc.vector.tensor_copy(
                    rwrows[r][:, tcn * 128:(tcn + 1) * 128], pr[:])

        # ================= Phase C: conv =================
        cwp = P("cwp", 2)
        cwt = P("cwt", 1)
        cps = P("cps", 3, "PSUM")
        wb1 = bcast_row(1, cwt, cps, "wb1")
        for oc in range(DC):
            wcv = cwp.tile([128, 32 * 128], BF16, tag="wcv", name="wcv")
            nc.sync.dma_start(
                wcv[:], wconv_d[:, oc * 32 * 128:(oc + 1) * 32 * 128])
            for (c0, cw) in cts:
                ps = cps.tile([128, 512], F32, tag="cpsum", name="cpsum")
                first = True
                for k in range(KC_):
                    for ic in range(DC):
                        nc.tensor.matmul(
                            ps[:, :cw],
                            wcv[:, (k * 8 + ic) * 128:(k * 8 + ic + 1) * 128],
                            h_s[:, ic * nt + c0 + k:ic * nt + c0 + k + cw],
                            start=first,
                            stop=(k == KC_ - 1 and ic == DC - 1))
                        first = False
                nc.vector.scalar_tensor_tensor(
                    out=c_s[:, oc * ntok + c0:oc * ntok + c0 + cw],
                    in0=ps[:, :cw], scalar=b_conv[:, oc:oc + 1],
                    in1=wb1[:, c0:c0 + cw], op0=ALU.add, op1=ALU.mult)
        rel(cwt, cwp, hp, cps, rp, rxp, hw, rps)

        # ================= Phase M: MoE (2 expert groups) ==========
        m1w = P("m1w", 2)
        m1t = P("m1t", 2)
        m1wb = P("m1wb", 2)
        m1ps = P("m1ps", 2, "PSUM")

        def moe_fc1(egrp, g_s):
            for el in range(4):
                e = egrp * 4 + el
                wbm = bcast_row(2 + e, m1wb, m1ps, "wbm")
                for j in range(4):
                    if j % 2 == 0:
                        we1 = m1w.tile([128, 32 * 128], BF16, tag="we1",
                                       name="we1")
                        nc.sync.dma_start(
                            we1[:],
                            we1_d[:, (e * 2 + j // 2) * 32 * 128:
                                  (e * 2 + j // 2 + 1) * 32 * 128])
                    for (c0, cw) in cts:
                        psa = m1ps.tile([128, 512], F32, tag="psa",
                                        name="psa")
                        psb = m1ps.tile([128, 512], F32, tag="psb",
                                        name="psb")
                        for ab, pst_ in ((0, psa), (1, psb)):
                            bi = ((j % 2) * 2 + ab) * 8
                            for kc in range(DC):
                                nc.tensor.matmul(
                                    pst_[:, :cw],
                                    we1[:, (bi + kc) * 128:
                                        (bi + kc + 1) * 128],
                                    x_s[:, kc * nt + HALO + c0:
                                        kc * nt + HALO + c0 + cw],
                                    start=(kc == 0), stop=(kc == DC - 1))
                        sg = m1t.tile([128, 512], BF16, tag="sg", name="sg")
                        nc.scalar.activation(
                            sg[:, :cw], psa[:, :cw], AF.Sigmoid,
                            bias=b_e1a[:, e * 4 + j:e * 4 + j + 1])
                        sa = m1t.tile([128, 512], BF16, tag="sa", name="sa")
                        nc.vector.scalar_tensor_tensor(
                            out=sa[:, :cw], in0=psa[:, :cw],
                            scalar=b_e1a[:, e * 4 + j:e * 4 + j + 1],
                            in1=sg[:, :cw], op0=ALU.add, op1=ALU.mult)
                        sa2 = m1t.tile([128, 512], BF16, tag="sa2",
                                       name="sa2")
                        nc.vector.tensor_mul(sa2[:, :cw], sa[:, :cw],
                                             wbm[:, c0:c0 + cw])
                        nc.vector.scalar_tensor_tensor(
                            out=g_s[:, (el * 4 + j) * ntok + c0:
                                    (el * 4 + j) * ntok + c0 + cw],
                            in0=psb[:, :cw],
                            scalar=b_e1b[:, e * 4 + j:e * 4 + j + 1],
                            in1=sa2[:, :cw], op0=ALU.add, op1=ALU.mult)

        def moe_fc2(egrp, g_s, m2ps, init):
            for mc in range(DC):
                we2 = m2w.tile([128, 16 * 128], BF16, tag="we2", name="we2")
                nc.sync.dma_start(
                    we2[:], we2_d[:, (egrp * 8 + mc) * 16 * 128:
                                  (egrp * 8 + mc + 1) * 16 * 128])
                for (c0, cw) in cts:
                    ps = m2ps.tile([128, 512], F32, tag="m2psum",
                                   name="m2psum")
                    for el in range(4):
                        for kc in range(4):
                            nc.tensor.matmul(
                                ps[:, :cw],
                                we2[:, (el * 4 + kc) * 128:
                                    (el * 4 + kc + 1) * 128],
                                g_s[:, (el * 4 + kc) * ntok + c0:
                                    (el * 4 + kc) * ntok + c0 + cw],
                                start=(el == 0 and kc == 0),
                                stop=(el == 3 and kc == 3))
                    nc.vector.tensor_add(
                        out_acc[:, mc * ntok + c0:mc * ntok + c0 + cw],
                        out_acc[:, mc * ntok + c0:mc * ntok + c0 + cw],
                        ps[:, :cw])

        gp0 = P("gp0", 1, side="right")
        g_s0 = gp0.tile([128, 16 * ntok], BF16, name="g_s0")
        moe_fc1(0, g_s0)

        # ============ Phase S: ssm out-proj + bias10 (init acc) =====
        sw = P("sw", 1)
        sps = P("sps", 3, "PSUM", side="right")
        wsout = sw.tile([128, 64 * 128], BF16)
        nc.sync.dma_start(wsout[:], wsout_d[:])
        for mc in range(DC):
            for (c0, cw) in cts:
                ps = sps.tile([128, 512], F32, tag="spsum", name="spsum")
                for kc in range(DC):
                    nc.tensor.matmul(
                        ps[:, :cw],
                        wsout[:, (mc * 8 + kc) * 128:(mc * 8 + kc + 1) * 128],
                        c_s[:, kc * ntok + c0:kc * ntok + c0 + cw],
                        start=(kc == 0), stop=False)
                nc.tensor.matmul(ps[:, :cw], b10[:, mc * 128:(mc + 1) * 128],
                                 rw10[:, c0:c0 + cw], start=False, stop=True)
                nc.scalar.copy(out_acc[:, mc * ntok + c0:mc * ntok + c0 + cw],
                               ps[:, :cw])

        rel(sw)
        rel(sps)
        m2w = P("m2w", 3)
        m2ps = P("m2ps", 3, "PSUM", side="right")
        gp1 = P("gp1", 1)
        g_s1 = gp1.tile([128, 16 * ntok], BF16, name="g_s1")
        moe_fc2(0, g_s0, m2ps, True)
        rel(gp0, cp)
        dw = P("dw", 2, side="right")
        dwb = P("dwb", 1, side="right")
        moe_fc1(1, g_s1)
        rel(m1ps)
        dps = P("dps", 2, "PSUM")
        wb0 = bcast_row(0, dwb, dps, "wb0")
        moe_fc2(1, g_s1, m2ps, False)

        # ================= Phase D: dense =================
        rel(gp1, m2w, m1wb, m1t, m1w, m2ps)
        d2w = P("d2w", 3)
        sap = P("sap", 1)
        dt_ = P("dt", 2)
        sa_s = sap.tile([128, 32 * ntok], BF16)
        for grp in range(4):
            wda = dw.tile([128, 64 * 128], BF16, tag="wd1", name="wda")
            nc.sync.dma_start(
                wda[:], wd1a_d[:, grp * 64 * 128:(grp + 1) * 64 * 128])
            for mcl in range(8):
                mc = grp * 8 + mcl
                for (c0, cw) in cts:
                    psa = dps.tile([128, 512], F32, tag="dpsa", name="dpsa")
                    for kc in range(DC):
                        nc.tensor.matmul(
                            psa[:, :cw],
                            wda[:, (mcl * 8 + kc) * 128:
                                (mcl * 8 + kc + 1) * 128],
                            x_s[:, kc * nt + HALO + c0:
                                kc * nt + HALO + c0 + cw],
                            start=(kc == 0), stop=(kc == DC - 1))
                    sg = dt_.tile([128, 512], BF16, tag="sg", name="sg")
                    nc.scalar.activation(sg[:, :cw], psa[:, :cw], AF.Sigmoid,
                                         bias=b_d1a[:, mc:mc + 1])
                    nc.vector.scalar_tensor_tensor(
                        out=sa_s[:, mc * ntok + c0:mc * ntok + c0 + cw],
                        in0=psa[:, :cw], scalar=b_d1a[:, mc:mc + 1],
                        in1=sg[:, :cw], op0=ALU.add, op1=ALU.mult)
        for grp in range(4):
            wdb = dw.tile([128, 64 * 128], BF16, tag="wd1", name="wdb")
            nc.sync.dma_start(
                wdb[:], wd1b_d[:, grp * 64 * 128:(grp + 1) * 64 * 128])
            for mcl in range(8):
                mc = grp * 8 + mcl
                for (c0, cw) in cts:
                    psb = dps.tile([128, 512], F32, tag="dpsb", name="dpsb")
                    for kc in range(DC):
                        nc.tensor.matmul(
                            psb[:, :cw],
                            wdb[:, (mcl * 8 + kc) * 128:
                                (mcl * 8 + kc + 1) * 128],
                            x_s[:, kc * nt + HALO + c0:
                                kc * nt + HALO + c0 + cw],
                            start=(kc == 0), stop=(kc == DC - 1))
                    hb = dt_.tile([128, 512], BF16, tag="hb", name="hb")
                    nc.scalar.activation(hb[:, :cw], psb[:, :cw],
                                         AF.Identity, bias=b_d1b[:, mc:mc + 1])
                    hb2 = dt_.tile([128, 512], BF16, tag="hb2", name="hb2")
                    nc.vector.tensor_mul(hb2[:, :cw], hb[:, :cw],
                                         wb0[:, c0:c0 + cw])
                    nc.vector.tensor_mul(
                        sa_s[:, mc * ntok + c0:mc * ntok + c0 + cw],
                        sa_s[:, mc * ntok + c0:mc * ntok + c0 + cw],
                        hb2[:, :cw])
        # dense fc2
        rel(dwb, dt_, dw, dps)
        d2ps = P("d2ps", 4, "PSUM")
        for mc in range(DC):
            for h in range(2):
                wd2 = d2w.tile([128, 16 * 128], BF16, tag="wd2", name="wd2")
                nc.sync.dma_start(
                    wd2[:], wd2_d[:, (h * 8 + mc) * 16 * 128:
                                  (h * 8 + mc + 1) * 16 * 128])
                for (c0, cw) in cts:
                    ps = d2ps.tile([128, 512], F32, tag="d2psum",
                                   name="d2psum")
                    for kc in range(16):
                        kg = h * 16 + kc
                        nc.tensor.matmul(
                            ps[:, :cw], wd2[:, kc * 128:(kc + 1) * 128],
                            sa_s[:, kg * ntok + c0:kg * ntok + c0 + cw],
                            start=(kc == 0), stop=(kc == 15))
                    nc.vector.tensor_add(
                        out_acc[:, mc * ntok + c0:mc * ntok + c0 + cw],
                        out_acc[:, mc * ntok + c0:mc * ntok + c0 + cw],
                        ps[:, :cw])
            for (c0, cw) in cts:
                nc.sync.dma_start(
                    out_d[:, mc * ntok + c0:mc * ntok + c0 + cw],
                    out_acc[:, mc * ntok + c0:mc * ntok + c0 + cw])
        for p in reversed(live):
            p.release()

    nc.compile()
    return nc



# ---------------- host-side packing ----------------

def _pack_km(WT, kcn, mcn):
    """WT [K, M] -> [128, kcn*mcn*128] with block idx = kc*mcn+mc."""
    return np.ascontiguousarray(
        WT.reshape(kcn, 128, mcn, 128).transpose(1, 0, 2, 3)
        .reshape(128, kcn * mcn * 128))


def _pack_mk(WT, kcn, mcn):
    """WT [K, M] -> [128, mcn*kcn*128] with block idx = mc*kcn+kc."""
    return np.ascontiguousarray(
        WT.reshape(kcn, 128, mcn, 128).transpose(1, 2, 0, 3)
        .reshape(128, mcn * kcn * 128))


def _featmajor(xt, ncols):
    """xt [1024, ncols] -> [128, 8*ncols] (kc-blocks along columns)."""
    return np.ascontiguousarray(
        xt.reshape(DC, 128, ncols).transpose(1, 0, 2).reshape(128, DC * ncols))


def _bias_cols(b, n):
    """b [n*128] -> [128, n] with col i = b[i*128:(i+1)*128]."""
    return np.ascontiguousarray(b.reshape(n, 128).T).astype(np.float32)


def pack_weights(rW, rb, d1W, d1b, d2W, d2b, sW_in, sb_in, sW_conv, sb_conv,
                 sW_out, sb_out, mW, mb, eW1, eb1, eW2, eb2):
    f32 = np.float32
    w = {}
    R = np.concatenate([rW.T, mW.T], axis=1).astype(f32)      # [1024, 11]
    Rh = R.astype(BF)
    Rl = (R - Rh.astype(f32)).astype(BF)
    w["w_rmh"] = _featmajor(Rh, 11)
    w["w_rml"] = _featmajor(Rl, 11)
    w["rm_bias"] = np.concatenate([rb, mb])[:, None].astype(f32)
    w["ident11"] = np.eye(11, dtype=f32)
    w["ident"] = np.eye(128, dtype=BF)
    w["ones1"] = np.ones((1, 128), dtype=BF)
    w["w_sin"] = _pack_mk(sW_in.T.astype(BF), 8, 8)
    w["b_sin"] = _bias_cols(sb_in, 8)
    # conv: A[k,i,o]; dst[p, ((oc*4+k)*8+ic)*128+c] = A[k, ic*128+p, oc*128+c]
    A = sW_conv.transpose(2, 1, 0).astype(BF)
    w["w_conv"] = np.ascontiguousarray(
        A.reshape(4, 8, 128, 8, 128).transpose(2, 3, 0, 1, 4)
        .reshape(128, 8 * 32 * 128))
    w["b_conv"] = _bias_cols(sb_conv, 8)
    w["w_sout"] = _pack_mk(sW_out.T.astype(BF), 8, 8)
    b10 = np.stack([d2b, sb_out] + [eW2b for eW2b in eb2], axis=0)
    w["b10"] = b10.astype(BF)                                  # [10, 1024]
    # experts fc1: block idx e*64 + (j*2+ab)*8 + kc ; m-chunk = ab*4+j
    morder = [ab * 4 + j for j in range(4) for ab in range(2)]
    slabs = []
    for e in range(E):
        Te = eW1[e].T.astype(BF).reshape(8, 128, 8, 128)      # kc,p,mc,c
        Te = Te[:, :, morder, :].transpose(1, 2, 0, 3)        # p,jm,kc,c
        slabs.append(Te.reshape(128, 64 * 128))
    w["w_e1"] = np.ascontiguousarray(np.concatenate(slabs, axis=1))
    eb1a = np.stack([eb1[e, j * 128:(j + 1) * 128]
                     for e in range(E) for j in range(4)], axis=1)
    eb1b = np.stack([eb1[e, 512 + j * 128: 512 + (j + 1) * 128]
                     for e in range(E) for j in range(4)], axis=1)
    w["b_e1a"] = eb1a.astype(f32)
    w["b_e1b"] = eb1b.astype(f32)
    # e2: col block ((egrp*8+mc)*16 + el*4 + kc), e = egrp*4+el
    T5 = np.stack([eW2[e].T.astype(BF).reshape(4, 128, 8, 128)
                   for e in range(E)])                        # e,kc,p,mc,c
    T6 = T5.reshape(2, 4, 4, 128, 8, 128)                     # g,el,kc,p,mc,c
    w["w_e2"] = np.ascontiguousarray(
        T6.transpose(3, 0, 4, 1, 2, 5).reshape(128, E * 32 * 128))
    w["w_d1a"] = _pack_mk(d1W[:HD].T.astype(BF), 8, 32)
    w["w_d1b"] = _pack_mk(d1W[HD:].T.astype(BF), 8, 32)
    w["b_d1a"] = _bias_cols(d1b[:HD], 32)
    w["b_d1b"] = _bias_cols(d1b[HD:], 32)
    # d2: block idx = h*128 + mc*16 + kcl, kg = h*16+kcl
    T4 = d2W.T.astype(BF).reshape(2, 16, 128, 8, 128)         # h,kcl,p,mc,c
    w["w_d2"] = np.ascontiguousarray(
        T4.transpose(2, 0, 3, 1, 4).reshape(128, 256 * 128))
    return w


def make_in_maps(x, weights, ntok=TOK, ncores=NCORE):
    """x [B,T,D] fp32 -> list of per-core in_maps."""
    xt = np.asarray(x, np.float32).reshape(-1, D).T           # [D, tokens]
    in_maps = []
    for c in range(ncores):
        lo = c * ntok
        xc = xt[:, lo:lo + ntok]
        halo = np.zeros((D, HALO), np.float32)
        if lo >= HALO and lo % T != 0:   # conv is causal per batch element
            halo = xt[:, lo - HALO:lo]
        xch = np.concatenate([halo, xc], axis=1)              # [D, nt]
        m = dict(weights)
        xh = xc.astype(BF)
        m["xl_s"] = _featmajor((xc - xh.astype(np.float32)).astype(BF), ntok)
        m["x_s"] = _featmajor(xch.astype(BF), ntok + HALO)
        in_maps.append(m)
    return in_maps


def assemble_output(results, ntok=TOK, ncores=NCORE):
    cols = []
    for c in range(ncores):
        o = results[c]["outT"]                                # [128, 8*ntok]
        cols.append(o.reshape(128, DC, ntok).transpose(1, 0, 2)
                    .reshape(D, ntok))
    full = np.concatenate(cols, axis=1)                       # [D, tokens]
    return np.ascontiguousarray(full.T).reshape(B, T, D).astype(np.float32)


_CACHED = {}


def kernel(**inputs):
    x = np.asarray(inputs["x"], np.float32)
    names = ["rW", "rb", "d1W", "d1b", "d2W", "d2b", "sW_in", "sb_in",
             "sW_conv", "sb_conv", "sW_out", "sb_out", "mW", "mb",
             "eW1", "eb1", "eW2", "eb2"]
    wargs = [np.asarray(inputs[n], np.float32) for n in names]
    if "nc" not in _CACHED:
        _CACHED["nc"] = build_program(TOK)
    nc = _CACHED["nc"]
    weights = pack_weights(*wargs)
    in_maps = make_in_maps(x, weights)
    res = bass_utils.run_bass_kernel_spmd(
        nc, in_maps, core_ids=list(range(NCORE)))
    return assemble_output(res.results)



# revision 9
# speedup vs baseline: 1.5030x; 1.5030x over previous
"""Trainium2 Bass kernel for nn_EvolutionBlock (moe_routing).

Strategy: data-parallel over the 8192 tokens across 8 NeuronCores
(1024 tokens/core + 3-token halo for the causal conv). Weights are
replicated per core and pre-packed on the host into the exact
[128, cols] SBUF layouts so every DMA is a contiguous slab.

On-chip everything is feature-major ([feature, token]) so matmuls are
out[f_chunk, tok] = lhsT.T @ rhs with lhsT = weight tile [din, dout]
and rhs = activation [din, tok]. Router/top-2 runs token-major in fp32
(selection must match the fp32 reference argmax), gets transposed via
the PE, and the per-token weights are broadcast across partitions with
K=1 ones-matmuls.

v2 speedups over the bf16 baseline:
 - The SSM branch is linear, so the in-proj, causal conv and out-proj
   collapse into 4 host-precomputed tap matrices
   N_k = sW_out @ sW_conv[:,:,k] @ sW_in; the conv phase consumes x
   directly and its PSUM result (scaled by the branch weight) IS the
   ssm contribution -> the h tile, in-proj and out-proj disappear.
 - The whole MoE branch runs in fp8e4m3 with DoubleRow (double-pumped)
   matmuls: expert weights are host-quantized at x64 scale, x at x1,
   and the swiglu output is re-quantized to fp8 at x16; the scales are
   folded into the activation scale, the ones-broadcast value (0.25)
   and the final 1/1024 accumulate. l2 error ~0.009 (gate 2e-2).
 - Dense swiglu uses the Silu activation directly (one scalar op
   instead of sigmoid+mul ops).
Branch combine weights are folded into the fc2 inputs so each branch's
final matmul accumulates the pre-scaled contribution; all branch
biases collapse into one [10, D] bias matmul against router-weight
rows.
"""

import numpy as np
import ml_dtypes

import concourse.bass as bass
import concourse.tile as tile
from concourse import bacc, mybir
from concourse import bass_utils

F32 = mybir.dt.float32
BF16 = mybir.dt.bfloat16
FP8 = mybir.dt.float8e4
AF = mybir.ActivationFunctionType
ALU = mybir.AluOpType
AX = mybir.AxisListType
DR = mybir.MatmulPerfMode.DoubleRow
BF = ml_dtypes.bfloat16
E4 = ml_dtypes.float8_e4m3

# Problem constants
B, T, D = 4, 2048, 1024
HD = 4096          # dense hidden (fc1 out = 2*HD)
S, KC_ = 1024, 4   # ssm state, conv kernel
E, HE = 8, 512     # experts, expert hidden
NCORE = 8
TOKENS = B * T
TOK = TOKENS // NCORE   # tokens per core
HALO = 3
DC = D // 128           # 8 d-chunks

WSCALE = 64.0      # expert weight quantization scale
GSCALE = 16.0      # expert swiglu-output quantization scale


def _coltiles(n, w=512):
    out = []
    c = 0
    while c < n:
        out.append((c, min(w, n - c)))
        c += w
    return out


def build_program(ntok=TOK):
    """Build + compile the Bass program for `ntok` tokens per core.

    Phase order: routers -> conv (writes out_acc) -> MoE -> dense.
    Pool lifetimes overlap across phase boundaries so the Tile
    scheduler can fill one phase's PE stalls with the next phase's
    matmuls; PSUM stays within 8 banks at every overlap point.
    """
    nt = ntok + HALO
    nc = bacc.Bacc("TRN2", target_bir_lowering=False, debug=False,
                   num_devices=NCORE)

    def din(name, shape, dt):
        return nc.dram_tensor(name, list(shape), dt, kind="ExternalInput").ap()

    xl_d = din("xl_s", [128, DC * ntok], BF16)
    xs_d = din("x_s", [128, DC * nt], BF16)
    x8_d = din("x8_s", [128, DC * ntok], FP8)
    wrmh_d = din("w_rmh", [128, DC * 11], BF16)
    wrml_d = din("w_rml", [128, DC * 11], BF16)
    rmb_d = din("rm_bias", [11, 1], F32)
    id11_d = din("ident11", [11, 11], F32)
    ident_d = din("ident", [128, 128], BF16)
    ones_d = din("ones1", [1, 128], BF16)
    oneq_d = din("oneq", [1, 128], BF16)
    wconv_d = din("w_conv", [128, 8 * 32 * 128], BF16)
    b10_d = din("b10", [10, 1024], BF16)
    we1_d = din("w_e1", [128, E * 64 * 128], FP8)
    be1a_d = din("b_e1a", [128, 32], F32)
    be1b_d = din("b_e1b", [128, 32], F32)
    we2_d = din("w_e2", [128, E * 32 * 128], FP8)
    wd1a_d = din("w_d1a", [128, 256 * 128], BF16)
    wd1b_d = din("w_d1b", [128, 256 * 128], BF16)
    bd1a_d = din("b_d1a", [128, 32], F32)
    bd1b_d = din("b_d1b", [128, 32], F32)
    wd2_d = din("w_d2", [128, 256 * 128], BF16)

    out_d = nc.dram_tensor("outT", [128, DC * ntok], F32,
                           kind="ExternalOutput").ap()

    cts = _coltiles(ntok)
    nchunk = ntok // 128

    with tile.TileContext(nc) as tc:
        live = []

        def P(name, bufs, space="SBUF", side="left"):
            p = tc.alloc_tile_pool(name=name, bufs=bufs, space=space,
                                   side=side)
            live.append(p)
            return p

        def rel(*ps):
            for p in ps:
                live.remove(p)
                p.release()

        constp = P("constp", 1)
        xp = P("xp", 1)
        x8p = P("x8p", 1)

        # Router pools first so the first-needed DMAs issue first.
        # (pbps outlives rps, so it is pushed below it on the right
        # PSUM stack.)
        pbps = P("pbps", 2, "PSUM", side="right")
        rxp = P("rxp", 1, side="right")
        rp = P("rp", nchunk, side="right")
        rps = P("rps", 1, "PSUM", side="right")

        x_s = xp.tile([128, DC * nt], BF16)
        nc.sync.dma_start(x_s[:], xs_d[:])
        xl_s = rxp.tile([128, DC * ntok], BF16)
        nc.sync.dma_start(xl_s[:], xl_d[:])
        wrmh = rxp.tile([128, DC * 11], BF16)
        nc.sync.dma_start(wrmh[:], wrmh_d[:])
        wrml = rxp.tile([128, DC * 11], BF16)
        nc.sync.dma_start(wrml[:], wrml_d[:])
        x8 = x8p.tile([128, DC * ntok], FP8)
        nc.sync.dma_start(x8[:], x8_d[:])
        x83 = x8.rearrange("p (b n) -> p b n", n=ntok)

        ident = constp.tile([128, 128], BF16)
        nc.sync.dma_start(ident[:], ident_d[:])
        ones1 = constp.tile([1, 128], BF16)
        nc.sync.dma_start(ones1[:], ones_d[:])
        oneq = constp.tile([1, 128], BF16)
        nc.sync.dma_start(oneq[:], oneq_d[:])
        rm_bias = constp.tile([11, 1], F32)
        nc.sync.dma_start(rm_bias[:], rmb_d[:])
        ident11 = constp.tile([11, 11], F32)
        nc.sync.dma_start(ident11[:], id11_d[:])
        b10 = constp.tile([10, 1024], BF16)
        nc.sync.dma_start(b10[:], b10_d[:])
        b_e1a = constp.tile([128, 32], F32)
        nc.sync.dma_start(b_e1a[:], be1a_d[:])
        b_e1b = constp.tile([128, 32], F32)
        nc.sync.dma_start(b_e1b[:], be1b_d[:])
        b_d1a = constp.tile([128, 32], F32)
        nc.sync.dma_start(b_d1a[:], bd1a_d[:])
        b_d1b = constp.tile([128, 32], F32)
        nc.sync.dma_start(b_d1b[:], bd1b_d[:])
        rw10 = constp.tile([10, ntok], BF16)
        rwrows = [constp.tile([1, ntok], BF16, tag=f"rwrow{r}",
                              name=f"rwrow{r}") for r in range(10)]
        out_acc = constp.tile([128, DC * ntok], F32)

        def bcast_row(r, pool, tag, ov):
            """[128, ntok] bf16 broadcast of ov*rw10 row r (K=1 matmul)."""
            wbt = pool.tile([128, ntok], BF16, tag=tag, name=tag)
            for (c0, cw) in cts:
                pb = pbps.tile([128, 512], F32, tag="pb", name="pb")
                nc.tensor.matmul(pb[:, :cw], ov[:],
                                 rwrows[r][:, c0:c0 + cw],
                                 start=True, stop=True)
                nc.scalar.copy(wbt[:, c0:c0 + cw], pb[:, :cw])
            return wbt

        # ================= Phase R: routers (stage-major) ==========
        rsbs, e3s, tm10s = [], [], []
        # stage 1: exact logits feature-major (3-term bf16 hi/lo),
        # then transpose [11,128]-chunks back to token-major
        lg = rxp.tile([11, ntok], F32, tag="lg", name="lg")
        for (c0, cw) in cts:
            ps = rps.tile([11, 512], F32, tag="ps", name="ps")
            nmm = 3 * DC
            im = 0
            for kc in range(DC):
                xh_c = x_s[:, kc * nt + HALO + c0:kc * nt + HALO + c0 + cw]
                xl_c = xl_s[:, kc * ntok + c0:kc * ntok + c0 + cw]
                wh_c = wrmh[:, kc * 11:(kc + 1) * 11]
                wl_c = wrml[:, kc * 11:(kc + 1) * 11]
                for (lhs_c, rhs_c) in ((wh_c, xh_c), (wl_c, xh_c),
                                       (wh_c, xl_c)):
                    nc.tensor.matmul(ps[:, :cw], lhs_c, rhs_c,
                                     start=(im == 0), stop=(im == nmm - 1))
                    im += 1
            nc.scalar.activation(lg[:, c0:c0 + cw], ps[:, :cw], AF.Identity,
                                 bias=rm_bias[:, 0:1])
        for tcn in range(nchunk):
            pst2 = rps.tile([128, 11], F32, tag="pst2", name="pst2")
            nc.tensor.transpose(pst2[:],
                                lg[:, tcn * 128:(tcn + 1) * 128], ident11[:])
            rsb = rp.tile([128, 11], F32, tag="rsb", name="rsb")
            nc.scalar.copy(rsb[:], pst2[:])
            e3 = rp.tile([128, 3], F32, tag="e3", name="e3")
            nc.scalar.activation(e3[:], rsb[:, 0:3], AF.Exp)
            rsbs.append(rsb)
            e3s.append(e3)
        # stage 2: top-2 + branch weights
        for tcn in range(nchunk):
            rsb, e3 = rsbs[tcn], e3s[tcn]
            s3 = rp.tile([128, 1], F32, tag="s3", name="s3")
            nc.vector.reduce_sum(s3[:], e3[:], axis=AX.X)
            r3 = rp.tile([128, 1], F32, tag="r3", name="r3")
            nc.vector.reciprocal(r3[:], s3[:])
            tm10 = rp.tile([128, 10], BF16, tag="tm10", name="tm10")
            nc.vector.tensor_scalar(out=tm10[:, 0:2], in0=e3[:, 0:2],
                                    scalar1=r3[:], scalar2=None, op0=ALU.mult)
            bw2 = rp.tile([128, 1], F32, tag="bw2", name="bw2")
            nc.vector.tensor_scalar(out=bw2[:], in0=e3[:, 2:3], scalar1=r3[:],
                                    scalar2=None, op0=ALU.mult)
            L = rsb[:, 3:11]
            m1 = rp.tile([128, 1], F32, tag="m1", name="m1")
            nc.vector.reduce_max(m1[:], L, axis=AX.X)
            mask1 = rp.tile([128, 8], F32, tag="mask1", name="mask1")
            nc.vector.tensor_scalar(out=mask1[:], in0=L, scalar1=m1[:],
                                    scalar2=None, op0=ALU.is_equal)
            L2 = rp.tile([128, 8], F32, tag="L2", name="L2")
            nc.vector.scalar_tensor_tensor(out=L2[:], in0=mask1[:],
                                           scalar=-1e9, in1=L,
                                           op0=ALU.mult, op1=ALU.add)
            m2 = rp.tile([128, 1], F32, tag="m2", name="m2")
            nc.vector.reduce_max(m2[:], L2[:], axis=AX.X)
            mask2 = rp.tile([128, 8], F32, tag="mask2", name="mask2")
            nc.vector.tensor_scalar(out=mask2[:], in0=L2[:], scalar1=m2[:],
                                    scalar2=None, op0=ALU.is_equal)
            dv = rp.tile([128, 1], F32, tag="dv", name="dv")
            nc.vector.tensor_sub(dv[:], m1[:], m2[:])
            w1 = rp.tile([128, 1], F32, tag="w1", name="w1")
            nc.scalar.activation(w1[:], dv[:], AF.Sigmoid)
            u1 = rp.tile([128, 1], F32, tag="u1", name="u1")
            nc.vector.tensor_mul(u1[:], w1[:], bw2[:])
            u2 = rp.tile([128, 1], F32, tag="u2", name="u2")
            nc.vector.tensor_sub(u2[:], bw2[:], u1[:])
            c2t = rp.tile([128, 8], F32, tag="c2t", name="c2t")
            nc.vector.tensor_scalar(out=c2t[:], in0=mask2[:], scalar1=u2[:],
                                    scalar2=None, op0=ALU.mult)
            nc.vector.scalar_tensor_tensor(out=tm10[:, 2:10], in0=mask1[:],
                                           scalar=u1[:], in1=c2t[:],
                                           op0=ALU.mult, op1=ALU.add)
            tm10s.append(tm10)
        # stage 3: transposes -> rw10 + per-row vectors
        for tcn in range(nchunk):
            tm10 = tm10s[tcn]
            pst = rps.tile([10, 128], BF16, tag="pst2", name="pst")
            nc.tensor.transpose(pst[:], tm10[:], ident[:])
            nc.scalar.copy(rw10[:, tcn * 128:(tcn + 1) * 128], pst[:])
            for r in range(10):
                pr = rps.tile([1, 128], BF16, tag="pr", name="pr", bufs=2)
                nc.tensor.transpose(pr[:], tm10[:, r:r + 1], ident[:])
                nc.vector.tensor_copy(
                    rwrows[r][:, tcn * 128:(tcn + 1) * 128], pr[:])

        rel(rps, rp, rxp)

        # ====== Phase C: fused ssm (conv of tap matrices), inits out_acc
        cwp = P("cwp", 2, side="right")
        cwt = P("cwt", 1, side="right")
        cps = P("cps", 2, "PSUM")
        wb1 = bcast_row(1, cwt, "wb1", ones1)
        for oc in range(DC):
            wcv = cwp.tile([128, 32 * 128], BF16, tag="wcv", name="wcv")
            nc.sync.dma_start(
                wcv[:], wconv_d[:, oc * 32 * 128:(oc + 1) * 32 * 128])
            for (c0, cw) in cts:
                ps = cps.tile([128, 512], F32, tag="cpsum", name="cpsum")
                first = True
                for k in range(KC_):
                    for ic in range(DC):
                        nc.tensor.matmul(
                            ps[:, :cw],
                            wcv[:, (k * 8 + ic) * 128:(k * 8 + ic + 1) * 128],
                            x_s[:, ic * nt + c0 + k:ic * nt + c0 + k + cw],
                            start=first,
                            stop=(k == KC_ - 1 and ic == DC - 1))
                        first = False
                nc.vector.tensor_mul(
                    out_acc[:, oc * ntok + c0:oc * ntok + c0 + cw],
                    ps[:, :cw], wb1[:, c0:c0 + cw])
        rel(cwt, cwp, cps)

        # ================= Phase M: MoE in fp8 (2 expert groups) ====
        m1w = P("m1w", 2, side="right")
        m1t = P("m1t", 2, side="right")
        m1wb = P("m1wb", 2, side="right")
        m1ps = P("m1ps", 2, "PSUM")

        def moe_fc1(egrp, g_s):
            for el in range(4):
                e = egrp * 4 + el
                wbm = bcast_row(2 + e, m1wb, "wbm", oneq)
                for j in range(4):
                    if j % 2 == 0:
                        we1 = m1w.tile([128, 32 * 128], FP8, tag="we1",
                                       name="we1")
                        nc.sync.dma_start(
                            we1[:],
                            we1_d[:, (e * 2 + j // 2) * 32 * 128:
                                  (e * 2 + j // 2 + 1) * 32 * 128])
                        w13 = we1.rearrange("p (b f) -> p b f", f=128)
                    bcol = e * 4 + j
                    for (c0, cw) in cts:
                        psa = m1ps.tile([128, 512], F32, tag="psa",
                                        name="psa")
                        psb = m1ps.tile([128, 512], F32, tag="psb",
                                        name="psb")
                        for ab, pst_ in ((0, psa), (1, psb)):
                            bi = ((j % 2) * 2 + ab) * 8
                            for t4 in range(4):
                                nc.tensor.matmul(
                                    pst_[:, :cw],
                                    w13[:, bi + 2 * t4:bi + 2 * t4 + 2, :],
                                    x83[:, 2 * t4:2 * t4 + 2, c0:c0 + cw],
                                    start=(t4 == 0), stop=(t4 == 3),
                                    perf_mode=DR)
                        sg = m1t.tile([128, 512], BF16, tag="sg", name="sg")
                        nc.scalar.activation(
                            sg[:, :cw], psa[:, :cw], AF.Silu,
                            bias=b_e1a[:, bcol:bcol + 1], scale=1.0 / WSCALE)
                        sa2 = m1t.tile([128, 512], BF16, tag="sa2",
                                       name="sa2")
                        nc.vector.tensor_mul(sa2[:, :cw], sg[:, :cw],
                                             wbm[:, c0:c0 + cw])
                        nc.vector.scalar_tensor_tensor(
                            out=g_s[:, (el * 4 + j) * ntok + c0:
                                    (el * 4 + j) * ntok + c0 + cw],
                            in0=psb[:, :cw],
                            scalar=b_e1b[:, bcol:bcol + 1],
                            in1=sa2[:, :cw], op0=ALU.add, op1=ALU.mult)

        def moe_fc2(egrp, g_s, m2ps):
            g3 = g_s.rearrange("p (b n) -> p b n", n=ntok)
            for mc in range(DC):
                we2 = m2w.tile([128, 16 * 128], FP8, tag="we2", name="we2")
                nc.sync.dma_start(
                    we2[:], we2_d[:, (egrp * 8 + mc) * 16 * 128:
                                  (egrp * 8 + mc + 1) * 16 * 128])
                w23 = we2.rearrange("p (b f) -> p b f", f=128)
                for (c0, cw) in cts:
                    ps = m2ps.tile([128, 512], F32, tag="m2psum",
                                   name="m2psum")
                    im = 0
                    for el in range(4):
                        for t2 in range(2):
                            blk = el * 4 + 2 * t2
                            nc.tensor.matmul(
                                ps[:, :cw],
                                w23[:, blk:blk + 2, :],
                                g3[:, blk:blk + 2, c0:c0 + cw],
                                start=(im == 0), stop=(im == 7),
                                perf_mode=DR)
                            im += 1
                    nc.vector.scalar_tensor_tensor(
                        out=out_acc[:, mc * ntok + c0:mc * ntok + c0 + cw],
                        in0=ps[:, :cw], scalar=1.0 / (WSCALE * GSCALE),
                        in1=out_acc[:, mc * ntok + c0:mc * ntok + c0 + cw],
                        op0=ALU.mult, op1=ALU.add)

        gp0 = P("gp0", 1, side="right")
        g_s0 = gp0.tile([128, 16 * ntok], FP8, name="g_s0")
        moe_fc1(0, g_s0)

        m2w = P("m2w", 3)
        m2ps = P("m2ps", 2, "PSUM", side="right")
        moe_fc2(0, g_s0, m2ps)
        rel(gp0)
        gp1 = P("gp1", 1, side="right")
        g_s1 = gp1.tile([128, 16 * ntok], FP8, name="g_s1")
        moe_fc1(1, g_s1)
        rel(m1ps)
        moe_fc2(1, g_s1, m2ps)
        rel(gp1, m1wb, m1t, m1w, m2ps, m2w)

        # ================= Phase D: dense =================
        sap = P("sap", 1)
        dw = P("dw", 2)
        dwb = P("dwb", 1)
        dt_ = P("dt", 2)
        dps = P("dps", 3, "PSUM")
        wb0 = bcast_row(0, dwb, "wb0", ones1)
        sa_s = sap.tile([128, 32 * ntok], BF16)
        for grp in range(4):
            wda = dw.tile([128, 64 * 128], BF16, tag="wd1", name="wda")
            nc.sync.dma_start(
                wda[:], wd1a_d[:, grp * 64 * 128:(grp + 1) * 64 * 128])
            for mcl in range(8):
                mc = grp * 8 + mcl
                for (c0, cw) in cts:
                    psa = dps.tile([128, 512], F32, tag="dpsa", name="dpsa")
                    for kc in range(DC):
                        nc.tensor.matmul(
                            psa[:, :cw],
                            wda[:, (mcl * 8 + kc) * 128:
                                (mcl * 8 + kc + 1) * 128],
                            x_s[:, kc * nt + HALO + c0:
                                kc * nt + HALO + c0 + cw],
                            start=(kc == 0), stop=(kc == DC - 1))
                    nc.scalar.activation(
                        sa_s[:, mc * ntok + c0:mc * ntok + c0 + cw],
                        psa[:, :cw], AF.Silu, bias=b_d1a[:, mc:mc + 1])
        for grp in range(4):
            wdb = dw.tile([128, 64 * 128], BF16, tag="wd1", name="wdb")
            nc.sync.dma_start(
                wdb[:], wd1b_d[:, grp * 64 * 128:(grp + 1) * 64 * 128])
            for mcl in range(8):
                mc = grp * 8 + mcl
                for (c0, cw) in cts:
                    psb = dps.tile([128, 512], F32, tag="dpsb", name="dpsb")
                    for kc in range(DC):
                        nc.tensor.matmul(
                            psb[:, :cw],
                            wdb[:, (mcl * 8 + kc) * 128:
                                (mcl * 8 + kc + 1) * 128],
                            x_s[:, kc * nt + HALO + c0:
                                kc * nt + HALO + c0 + cw],
                            start=(kc == 0), stop=(kc == DC - 1))
                    hb2 = dt_.tile([128, 512], BF16, tag="hb2", name="hb2")
                    nc.vector.scalar_tensor_tensor(
                        out=hb2[:, :cw], in0=psb[:, :cw],
                        scalar=b_d1b[:, mc:mc + 1],
                        in1=wb0[:, c0:c0 + cw], op0=ALU.add, op1=ALU.mult)
                    nc.vector.tensor_mul(
                        sa_s[:, mc * ntok + c0:mc * ntok + c0 + cw],
                        sa_s[:, mc * ntok + c0:mc * ntok + c0 + cw],
                        hb2[:, :cw])
        # dense fc2 (+ the collapsed bias matmul in the h==0 chain)
        rel(dps, dt_, dwb, dw)
        d2w = P("d2w", 3)
        d2ps = P("d2ps", 4, "PSUM")
        for mc in range(DC):
            for h in range(2):
                wd2 = d2w.tile([128, 16 * 128], BF16, tag="wd2", name="wd2")
                nc.sync.dma_start(
                    wd2[:], wd2_d[:, (h * 8 + mc) * 16 * 128:
                                  (h * 8 + mc + 1) * 16 * 128])
                for (c0, cw) in cts:
                    ps = d2ps.tile([128, 512], F32, tag="d2psum",
                                   name="d2psum")
                    for kc in range(16):
                        kg = h * 16 + kc
                        nc.tensor.matmul(
                            ps[:, :cw], wd2[:, kc * 128:(kc + 1) * 128],
                            sa_s[:, kg * ntok + c0:kg * ntok + c0 + cw],
                            start=(kc == 0), stop=(h == 1 and kc == 15))
                    if h == 0:
                        nc.tensor.matmul(
                            ps[:, :cw], b10[:, mc * 128:(mc + 1) * 128],
                            rw10[:, c0:c0 + cw], start=False, stop=True)
                    nc.vector.tensor_add(
                        out_acc[:, mc * ntok + c0:mc * ntok + c0 + cw],
                        out_acc[:, mc * ntok + c0:mc * ntok + c0 + cw],
                        ps[:, :cw])
            for (c0, cw) in cts:
                nc.sync.dma_start(
                    out_d[:, mc * ntok + c0:mc * ntok + c0 + cw],
                    out_acc[:, mc * ntok + c0:mc * ntok + c0 + cw])
        for p in reversed(live):
            p.release()

    nc.compile()
    return nc


# ---------------- host-side packing ----------------

def _pack_mk(WT, kcn, mcn):
    """WT [K, M] -> [128, mcn*kcn*128] with block idx = mc*kcn+kc."""
    return np.ascontiguousarray(
        WT.reshape(kcn, 128, mcn, 128).transpose(1, 2, 0, 3)
        .reshape(128, mcn * kcn * 128))


def _featmajor(xt, ncols):
    """xt [1024, ncols] -> [128, 8*ncols] (kc-blocks along columns)."""
    return np.ascontiguousarray(
        xt.reshape(DC, 128, ncols).transpose(1, 0, 2).reshape(128, DC * ncols))


def _bias_cols(b, n):
    """b [n*128] -> [128, n] with col i = b[i*128:(i+1)*128]."""
    return np.ascontiguousarray(b.reshape(n, 128).T).astype(np.float32)


def _q8(a, scale):
    return np.clip(np.asarray(a, np.float64) * scale,
                   -240.0, 240.0).astype(E4)


def pack_weights(rW, rb, d1W, d1b, d2W, d2b, sW_in, sb_in, sW_conv, sb_conv,
                 sW_out, sb_out, mW, mb, eW1, eb1, eW2, eb2):
    f32 = np.float32
    w = {}
    R = np.concatenate([rW.T, mW.T], axis=1).astype(f32)      # [1024, 11]
    Rh = R.astype(BF)
    Rl = (R - Rh.astype(f32)).astype(BF)
    w["w_rmh"] = _featmajor(Rh, 11)
    w["w_rml"] = _featmajor(Rl, 11)
    w["rm_bias"] = np.concatenate([rb, mb])[:, None].astype(f32)
    w["ident11"] = np.eye(11, dtype=f32)
    w["ident"] = np.eye(128, dtype=BF)
    w["ones1"] = np.ones((1, 128), dtype=BF)
    w["oneq"] = np.full((1, 128), GSCALE / WSCALE, BF)
    # fused ssm taps: N_k = sW_out @ sW_conv[:,:,k] @ sW_in, packed like
    # the conv layout: dst[p, ((oc*4+k)*8+ic)*128+c] = N_k[oc*128+c, ic*128+p]
    sW_out64 = sW_out.astype(np.float64)
    sW_in64 = sW_in.astype(np.float64)
    Nk = np.stack([(sW_out64 @ sW_conv[:, :, k].astype(np.float64)
                    @ sW_in64).T for k in range(KC_)], axis=0)  # [k, i, o]
    A = Nk.astype(BF)
    w["w_conv"] = np.ascontiguousarray(
        A.reshape(4, 8, 128, 8, 128).transpose(2, 3, 0, 1, 4)
        .reshape(128, 8 * 32 * 128))
    # collapsed branch biases: row0 dense, row1 full ssm bias, rows2-9 moe
    b_ssm = (sW_out64 @ (sW_conv.astype(np.float64).sum(axis=2)
                         @ sb_in.astype(np.float64)
                         + sb_conv.astype(np.float64))
             + sb_out.astype(np.float64)).astype(f32)
    b10 = np.stack([d2b, b_ssm] + [eW2b for eW2b in eb2], axis=0)
    w["b10"] = b10.astype(BF)                                  # [10, 1024]
    # experts fc1 (fp8 x WSCALE): block idx e*64 + (j*2+ab)*8 + kc
    morder = [ab * 4 + j for j in range(4) for ab in range(2)]
    slabs = []
    for e in range(E):
        Te = _q8(eW1[e].T, WSCALE).reshape(8, 128, 8, 128)    # kc,p,mc,c
        Te = Te[:, :, morder, :].transpose(1, 2, 0, 3)        # p,jm,kc,c
        slabs.append(Te.reshape(128, 64 * 128))
    w["w_e1"] = np.ascontiguousarray(np.concatenate(slabs, axis=1))
    eb1a = np.stack([eb1[e, j * 128:(j + 1) * 128]
                     for e in range(E) for j in range(4)], axis=1)
    eb1b = np.stack([eb1[e, 512 + j * 128: 512 + (j + 1) * 128]
                     for e in range(E) for j in range(4)], axis=1)
    w["b_e1a"] = eb1a.astype(f32)
    w["b_e1b"] = (eb1b * WSCALE).astype(f32)
    # e2 (fp8 x WSCALE): col block ((egrp*8+mc)*16 + el*4 + kc), e=egrp*4+el
    T5 = np.stack([_q8(eW2[e].T, WSCALE).reshape(4, 128, 8, 128)
                   for e in range(E)])                        # e,kc,p,mc,c
    T6 = T5.reshape(2, 4, 4, 128, 8, 128)                     # g,el,kc,p,mc,c
    w["w_e2"] = np.ascontiguousarray(
        T6.transpose(3, 0, 4, 1, 2, 5).reshape(128, E * 32 * 128))
    w["w_d1a"] = _pack_mk(d1W[:HD].T.astype(BF), 8, 32)
    w["w_d1b"] = _pack_mk(d1W[HD:].T.astype(BF), 8, 32)
    w["b_d1a"] = _bias_cols(d1b[:HD], 32)
    w["b_d1b"] = _bias_cols(d1b[HD:], 32)
    # d2: block idx = h*128 + mc*16 + kcl, kg = h*16+kcl
    T4 = d2W.T.astype(BF).reshape(2, 16, 128, 8, 128)         # h,kcl,p,mc,c
    w["w_d2"] = np.ascontiguousarray(
        T4.transpose(2, 0, 3, 1, 4).reshape(128, 256 * 128))
    return w


def make_in_maps(x, weights, ntok=TOK, ncores=NCORE):
    """x [B,T,D] fp32 -> list of per-core in_maps."""
    xt = np.asarray(x, np.float32).reshape(-1, D).T           # [D, tokens]
    in_maps = []
    for c in range(ncores):
        lo = c * ntok
        xc = xt[:, lo:lo + ntok]
        halo = np.zeros((D, HALO), np.float32)
        if lo >= HALO and lo % T != 0:   # conv is causal per batch element
            halo = xt[:, lo - HALO:lo]
        xch = np.concatenate([halo, xc], axis=1)              # [D, nt]
        m = dict(weights)
        xh = xc.astype(BF)
        m["xl_s"] = _featmajor((xc - xh.astype(np.float32)).astype(BF), ntok)
        m["x_s"] = _featmajor(xch.astype(BF), ntok + HALO)
        m["x8_s"] = _featmajor(np.clip(xc, -240.0, 240.0), ntok).astype(E4)
        in_maps.append(m)
    return in_maps


def assemble_output(results, ntok=TOK, ncores=NCORE):
    cols = []
    for c in range(ncores):
        o = results[c]["outT"]                                # [128, 8*ntok]
        cols.append(o.reshape(128, DC, ntok).transpose(1, 0, 2)
                    .reshape(D, ntok))
    full = np.concatenate(cols, axis=1)                       # [D, tokens]
    return np.ascontiguousarray(full.T).reshape(B, T, D).astype(np.float32)


_CACHED = {}


def kernel(**inputs):
    x = np.asarray(inputs["x"], np.float32)
    names = ["rW", "rb", "d1W", "d1b", "d2W", "d2b", "sW_in", "sb_in",
             "sW_conv", "sb_conv", "sW_out", "sb_out", "mW", "mb",
             "eW1", "eb1", "eW2", "eb2"]
    wargs = [np.asarray(inputs[n], np.float32) for n in names]
    if "nc" not in _CACHED:
        _CACHED["nc"] = build_program(TOK)
    nc = _CACHED["nc"]
    weights = pack_weights(*wargs)
    in_maps = make_in_maps(x, weights)
    res = bass_utils.run_bass_kernel_spmd(
        nc, in_maps, core_ids=list(range(NCORE)))
    return assemble_output(res.results)


# revision 11
# speedup vs baseline: 1.5710x; 1.0452x over previous
"""Trainium2 Bass kernel for nn_EvolutionBlock (moe_routing).

Strategy: data-parallel over the 8192 tokens across 8 NeuronCores
(1024 tokens/core + 3-token halo for the causal conv). Weights are
replicated per core and pre-packed on the host into the exact
[128, cols] SBUF layouts so every DMA is a contiguous slab.

On-chip everything is feature-major ([feature, token]) so matmuls are
out[f_chunk, tok] = lhsT.T @ rhs with lhsT = weight tile [din, dout]
and rhs = activation [din, tok]. Router/top-2 runs token-major in fp32
(selection must match the fp32 reference argmax), gets transposed via
the PE, and the per-token branch weights are broadcast across
partitions with one-hot-row selector matmuls against the [10, ntok]
weight matrix.

Speedups over the bf16 baseline:
 - The SSM branch is linear, so the in-proj, causal conv and out-proj
   collapse into 4 host-precomputed tap matrices
   N_k = sW_out @ sW_conv[:,:,k] @ sW_in; the conv phase consumes x
   directly and its PSUM result (scaled by the branch weight) IS the
   ssm contribution.
 - The whole MoE branch runs in fp8e4m3 with DoubleRow (double-pumped)
   matmuls: expert weights are host-quantized at x64 scale, x at x1,
   and the swiglu output is re-quantized to fp8 at x16; the scales are
   folded into the activation scale, the selector value (0.25)
   and the final 1/1024 accumulate. l2 error ~0.009 (gate 2e-2).
 - Dense swiglu uses the Silu activation directly.
 - Phases are ordered R, C, fc1(all experts), fc2(all), D1a, D1b, D2
   so no PE chain ever waits on its own phase's drain.
Branch combine weights are folded into the fc2 inputs so each branch's
final matmul accumulates the pre-scaled contribution; all branch
biases collapse into one [10, D] bias matmul against router-weight
rows, appended to the dense-fc2 PSUM chains.
"""

import numpy as np
import ml_dtypes

import concourse.bass as bass
import concourse.tile as tile
from concourse import bacc, mybir
from concourse import bass_utils

F32 = mybir.dt.float32
BF16 = mybir.dt.bfloat16
FP8 = mybir.dt.float8e4
AF = mybir.ActivationFunctionType
ALU = mybir.AluOpType
AX = mybir.AxisListType
DR = mybir.MatmulPerfMode.DoubleRow
BF = ml_dtypes.bfloat16
E4 = ml_dtypes.float8_e4m3

# Problem constants
B, T, D = 4, 2048, 1024
HD = 4096          # dense hidden (fc1 out = 2*HD)
S, KC_ = 1024, 4   # ssm state, conv kernel
E, HE = 8, 512     # experts, expert hidden (swiglu)
NCORE = 8
TOKENS = B * T
TOK = TOKENS // NCORE   # tokens per core
HALO = 3
DC = D // 128           # 8 d-chunks

WSCALE = 64.0      # expert weight quantization scale
GSCALE = 16.0      # expert swiglu-output quantization scale


def _coltiles(n, w=512):
    out = []
    c = 0
    while c < n:
        out.append((c, min(w, n - c)))
        c += w
    return out


def build_program(ntok=TOK):
    """Build + compile the Bass program for `ntok` tokens per core."""
    nt = ntok + HALO
    nc = bacc.Bacc("TRN2", target_bir_lowering=False, debug=False,
                   num_devices=NCORE)

    def din(name, shape, dt):
        return nc.dram_tensor(name, list(shape), dt, kind="ExternalInput").ap()

    xl_d = din("xl_s", [128, DC * ntok], BF16)
    xs_d = din("x_s", [128, DC * nt], BF16)
    x8_d = din("x8_s", [128, DC * ntok], FP8)
    wrmh_d = din("w_rmh", [128, DC * 11], BF16)
    wrml_d = din("w_rml", [128, DC * 11], BF16)
    rmb_d = din("rm_bias", [11, 1], F32)
    id11_d = din("ident11", [11, 11], F32)
    ident_d = din("ident", [128, 128], BF16)
    sel10_d = din("sel10", [10, 10 * 128], BF16)
    wconv_d = din("w_conv", [128, 8 * 32 * 128], BF16)
    b10_d = din("b10", [10, 1024], BF16)
    we1_d = din("w_e1", [128, E * 64 * 128], FP8)
    be1a_d = din("b_e1a", [128, 32], F32)
    be1b_d = din("b_e1b", [128, 32], F32)
    we2_d = din("w_e2", [128, E * 32 * 128], FP8)
    wd1a_d = din("w_d1a", [128, 256 * 128], BF16)
    wd1b_d = din("w_d1b", [128, 256 * 128], BF16)
    bd1a_d = din("b_d1a", [128, 32], F32)
    bd1b_d = din("b_d1b", [128, 32], F32)
    wd2_d = din("w_d2", [128, 256 * 128], BF16)

    out_d = nc.dram_tensor("outT", [128, DC * ntok], F32,
                           kind="ExternalOutput").ap()

    cts = _coltiles(ntok)
    nchunk = ntok // 128

    with tile.TileContext(nc) as tc:
        live = []

        def P(name, bufs, space="SBUF", side="left"):
            p = tc.alloc_tile_pool(name=name, bufs=bufs, space=space,
                                   side=side)
            live.append(p)
            return p

        def rel(*ps):
            for p in ps:
                live.remove(p)
                p.release()

        constp = P("constp", 1)
        xp = P("xp", 1)
        x8p = P("x8p", 1)

        # Router pools first so the first-needed DMAs issue first.
        pbps = P("pbps", 2, "PSUM", side="right")
        rxp = P("rxp", 1, side="right")
        rp = P("rp", nchunk, side="right")
        rps = P("rps", 1, "PSUM", side="right")

        # x/xl arrive in column halves so the first router chain can
        # start after ~1/4 of the activation DMA bytes.
        x_s = xp.tile([128, DC * nt], BF16)
        xl_s = rxp.tile([128, DC * ntok], BF16)
        xs3s = x_s.rearrange("p (k n) -> p k n", n=nt)
        xs3d = xs_d.rearrange("p (k n) -> p k n", n=nt)
        xl3s = xl_s.rearrange("p (k n) -> p k n", n=ntok)
        xl3d = xl_d.rearrange("p (k n) -> p k n", n=ntok)
        wrmh = rxp.tile([128, DC * 11], BF16)
        nc.sync.dma_start(wrmh[:], wrmh_d[:])
        wrml = rxp.tile([128, DC * 11], BF16)
        nc.sync.dma_start(wrml[:], wrml_d[:])
        nc.sync.dma_start(xs3s[:, :, 0:516], xs3d[:, :, 0:516])
        nc.sync.dma_start(xl3s[:, :, 0:512], xl3d[:, :, 0:512])
        nc.sync.dma_start(xs3s[:, :, 516:nt], xs3d[:, :, 516:nt])
        nc.sync.dma_start(xl3s[:, :, 512:ntok], xl3d[:, :, 512:ntok])
        x8 = x8p.tile([128, DC * ntok], FP8)
        nc.sync.dma_start(x8[:], x8_d[:])
        x83 = x8.rearrange("p (b n) -> p b n", n=ntok)

        ident = constp.tile([128, 128], BF16)
        nc.sync.dma_start(ident[:], ident_d[:])
        sel10 = constp.tile([10, 10 * 128], BF16)
        nc.sync.dma_start(sel10[:], sel10_d[:])
        rm_bias = constp.tile([11, 1], F32)
        nc.sync.dma_start(rm_bias[:], rmb_d[:])
        ident11 = constp.tile([11, 11], F32)
        nc.sync.dma_start(ident11[:], id11_d[:])
        b10 = constp.tile([10, 1024], BF16)
        nc.sync.dma_start(b10[:], b10_d[:])
        b_e1a = constp.tile([128, 32], F32)
        nc.sync.dma_start(b_e1a[:], be1a_d[:])
        b_e1b = constp.tile([128, 32], F32)
        nc.sync.dma_start(b_e1b[:], be1b_d[:])
        b_d1a = constp.tile([128, 32], F32)
        nc.sync.dma_start(b_d1a[:], bd1a_d[:])
        b_d1b = constp.tile([128, 32], F32)
        nc.sync.dma_start(b_d1b[:], bd1b_d[:])
        rw10 = constp.tile([10, ntok], BF16)
        out_acc = constp.tile([128, DC * ntok], F32)

        def bcast_row(r, pool, tag):
            """[128, ntok] bf16 broadcast of sel_r * rw10 row r (K=10)."""
            wbt = pool.tile([128, ntok], BF16, tag=tag, name=tag)
            for (c0, cw) in cts:
                pb = pbps.tile([128, 512], F32, tag="pb", name="pb")
                nc.tensor.matmul(pb[:, :cw],
                                 sel10[:, r * 128:(r + 1) * 128],
                                 rw10[:, c0:c0 + cw],
                                 start=True, stop=True)
                nc.scalar.copy(wbt[:, c0:c0 + cw], pb[:, :cw])
            return wbt

        # ================= Phase R: routers (stage-major) ==========
        rsbs, e3s, tm10s = [], [], []
        # stage 1: exact logits feature-major (3-term bf16 hi/lo); the
        # xh-only terms run first so the xl DMA overlaps them.
        lg = rxp.tile([11, ntok], F32, tag="lg", name="lg")
        for (c0, cw) in cts:
            ps = rps.tile([11, 512], F32, tag="ps", name="ps")
            nmm = 3 * DC
            im = 0
            for (lhs_w, rhs_x) in (("h", "h"), ("l", "h"), ("h", "l")):
                for kc in range(DC):
                    lhs_c = (wrmh if lhs_w == "h" else
                             wrml)[:, kc * 11:(kc + 1) * 11]
                    if rhs_x == "h":
                        rhs_c = x_s[:, kc * nt + HALO + c0:
                                    kc * nt + HALO + c0 + cw]
                    else:
                        rhs_c = xl_s[:, kc * ntok + c0:kc * ntok + c0 + cw]
                    nc.tensor.matmul(ps[:, :cw], lhs_c, rhs_c,
                                     start=(im == 0), stop=(im == nmm - 1))
                    im += 1
            nc.scalar.activation(lg[:, c0:c0 + cw], ps[:, :cw], AF.Identity,
                                 bias=rm_bias[:, 0:1])
        for tcn in range(nchunk):
            pst2 = rps.tile([128, 11], F32, tag="pst2", name="pst2")
            nc.tensor.transpose(pst2[:],
                                lg[:, tcn * 128:(tcn + 1) * 128], ident11[:])
            rsb = rp.tile([128, 11], F32, tag="rsb", name="rsb")
            nc.vector.tensor_copy(rsb[:], pst2[:])
            e3 = rp.tile([128, 3], F32, tag="e3", name="e3")
            nc.scalar.activation(e3[:], rsb[:, 0:3], AF.Exp)
            rsbs.append(rsb)
            e3s.append(e3)
        # stage 2: top-2 + branch weights
        for tcn in range(nchunk):
            rsb, e3 = rsbs[tcn], e3s[tcn]
            s3 = rp.tile([128, 1], F32, tag="s3", name="s3")
            nc.vector.reduce_sum(s3[:], e3[:], axis=AX.X)
            r3 = rp.tile([128, 1], F32, tag="r3", name="r3")
            nc.vector.reciprocal(r3[:], s3[:])
            tm10 = rp.tile([128, 10], BF16, tag="tm10", name="tm10")
            nc.vector.tensor_scalar(out=tm10[:, 0:2], in0=e3[:, 0:2],
                                    scalar1=r3[:], scalar2=None, op0=ALU.mult)
            bw2 = rp.tile([128, 1], F32, tag="bw2", name="bw2")
            nc.vector.tensor_scalar(out=bw2[:], in0=e3[:, 2:3], scalar1=r3[:],
                                    scalar2=None, op0=ALU.mult)
            L = rsb[:, 3:11]
            m1 = rp.tile([128, 1], F32, tag="m1", name="m1")
            nc.vector.reduce_max(m1[:], L, axis=AX.X)
            mask1 = rp.tile([128, 8], F32, tag="mask1", name="mask1")
            nc.vector.tensor_scalar(out=mask1[:], in0=L, scalar1=m1[:],
                                    scalar2=None, op0=ALU.is_equal)
            L2 = rp.tile([128, 8], F32, tag="L2", name="L2")
            nc.vector.scalar_tensor_tensor(out=L2[:], in0=mask1[:],
                                           scalar=-1e9, in1=L,
                                           op0=ALU.mult, op1=ALU.add)
            m2 = rp.tile([128, 1], F32, tag="m2", name="m2")
            nc.vector.reduce_max(m2[:], L2[:], axis=AX.X)
            mask2 = rp.tile([128, 8], F32, tag="mask2", name="mask2")
            nc.vector.tensor_scalar(out=mask2[:], in0=L2[:], scalar1=m2[:],
                                    scalar2=None, op0=ALU.is_equal)
            dv = rp.tile([128, 1], F32, tag="dv", name="dv")
            nc.vector.tensor_sub(dv[:], m1[:], m2[:])
            w1 = rp.tile([128, 1], F32, tag="w1", name="w1")
            nc.scalar.activation(w1[:], dv[:], AF.Sigmoid)
            u1 = rp.tile([128, 1], F32, tag="u1", name="u1")
            nc.vector.tensor_mul(u1[:], w1[:], bw2[:])
            u2 = rp.tile([128, 1], F32, tag="u2", name="u2")
            nc.vector.tensor_sub(u2[:], bw2[:], u1[:])
            c2t = rp.tile([128, 8], F32, tag="c2t", name="c2t")
            nc.vector.tensor_scalar(out=c2t[:], in0=mask2[:], scalar1=u2[:],
                                    scalar2=None, op0=ALU.mult)
            nc.vector.scalar_tensor_tensor(out=tm10[:, 2:10], in0=mask1[:],
                                           scalar=u1[:], in1=c2t[:],
                                           op0=ALU.mult, op1=ALU.add)
            tm10s.append(tm10)
        # stage 3: transpose back to the [10, ntok] weight matrix
        for tcn in range(nchunk):
            pst = rps.tile([10, 128], BF16, tag="pst2", name="pst")
            nc.tensor.transpose(pst[:], tm10s[tcn][:], ident[:])
            nc.vector.tensor_copy(rw10[:, tcn * 128:(tcn + 1) * 128], pst[:])
        rel(rps, rp, rxp)

        # ====== Phase C: fused ssm (conv of tap matrices), inits out_acc
        cwp = P("cwp", 2, side="right")
        cwt = P("cwt", 1, side="right")
        cps = P("cps", 3, "PSUM")
        wb1 = bcast_row(1, cwt, "wb1")
        for oc in range(DC):
            wcv = cwp.tile([128, 32 * 128], BF16, tag="wcv", name="wcv")
            nc.sync.dma_start(
                wcv[:], wconv_d[:, oc * 32 * 128:(oc + 1) * 32 * 128])
            for (c0, cw) in cts:
                ps = cps.tile([128, 512], F32, tag="cpsum", name="cpsum")
                first = True
                for k in range(KC_):
                    for ic in range(DC):
                        nc.tensor.matmul(
                            ps[:, :cw],
                            wcv[:, (k * 8 + ic) * 128:(k * 8 + ic + 1) * 128],
                            x_s[:, ic * nt + c0 + k:ic * nt + c0 + k + cw],
                            start=first,
                            stop=(k == KC_ - 1 and ic == DC - 1))
                        first = False
                nc.vector.tensor_mul(
                    out_acc[:, oc * ntok + c0:oc * ntok + c0 + cw],
                    ps[:, :cw], wb1[:, c0:c0 + cw])
        rel(cwt, cwp, cps)

        # ================= Phase M: MoE in fp8 =====================
        gp0 = P("gp0", 1, side="right")
        gp1 = P("gp1", 1, side="right")
        m1w = P("m1w", 2, side="right")
        m1t = P("m1t", 2, side="right")
        m1wb = P("m1wb", 2, side="right")
        m1ps = P("m1ps", 2, "PSUM")
        g_s0 = gp0.tile([128, 16 * ntok], FP8, name="g_s0")
        g_s1 = gp1.tile([128, 16 * ntok], FP8, name="g_s1")

        for e in range(E):
            g_s = g_s0 if e < 4 else g_s1
            el = e % 4
            wbm = bcast_row(2 + e, m1wb, "wbm")
            for j in range(4):
                if j % 2 == 0:
                    we1 = m1w.tile([128, 32 * 128], FP8, tag="we1",
                                   name="we1")
                    nc.sync.dma_start(
                        we1[:],
                        we1_d[:, (e * 2 + j // 2) * 32 * 128:
                              (e * 2 + j // 2 + 1) * 32 * 128])
                    w13 = we1.rearrange("p (b f) -> p b f", f=128)
                bcol = e * 4 + j
                for (c0, cw) in cts:
                    psa = m1ps.tile([128, 512], F32, tag="psa", name="psa")
                    psb = m1ps.tile([128, 512], F32, tag="psb", name="psb")
                    for ab, pst_ in ((0, psa), (1, psb)):
                        bi = ((j % 2) * 2 + ab) * 8
                        for t4 in range(4):
                            nc.tensor.matmul(
                                pst_[:, :cw],
                                w13[:, bi + 2 * t4:bi + 2 * t4 + 2, :],
                                x83[:, 2 * t4:2 * t4 + 2, c0:c0 + cw],
                                start=(t4 == 0), stop=(t4 == 3),
                                perf_mode=DR)
                    sg = m1t.tile([128, 512], BF16, tag="sg", name="sg")
                    nc.scalar.activation(
                        sg[:, :cw], psa[:, :cw], AF.Silu,
                        bias=b_e1a[:, bcol:bcol + 1], scale=1.0 / WSCALE)
                    sa2 = m1t.tile([128, 512], BF16, tag="sa2", name="sa2")
                    nc.vector.tensor_mul(sa2[:, :cw], sg[:, :cw],
                                         wbm[:, c0:c0 + cw])
                    nc.vector.scalar_tensor_tensor(
                        out=g_s[:, (el * 4 + j) * ntok + c0:
                                (el * 4 + j) * ntok + c0 + cw],
                        in0=psb[:, :cw],
                        scalar=b_e1b[:, bcol:bcol + 1],
                        in1=sa2[:, :cw], op0=ALU.add, op1=ALU.mult)
        rel(m1ps, m1wb, m1t, m1w)

        m2w = P("m2w", 3)
        m2ps = P("m2ps", 3, "PSUM", side="right")
        g30 = g_s0.rearrange("p (b n) -> p b n", n=ntok)
        g31 = g_s1.rearrange("p (b n) -> p b n", n=ntok)
        for egrp in range(2):
            g3 = g30 if egrp == 0 else g31
            for mc in range(DC):
                we2 = m2w.tile([128, 16 * 128], FP8, tag="we2", name="we2")
                nc.sync.dma_start(
                    we2[:], we2_d[:, (egrp * 8 + mc) * 16 * 128:
                                  (egrp * 8 + mc + 1) * 16 * 128])
                w23 = we2.rearrange("p (b f) -> p b f", f=128)
                for (c0, cw) in cts:
                    ps = m2ps.tile([128, 512], F32, tag="m2psum",
                                   name="m2psum")
                    im = 0
                    for el in range(4):
                        for t2 in range(2):
                            blk = el * 4 + 2 * t2
                            nc.tensor.matmul(
                                ps[:, :cw],
                                w23[:, blk:blk + 2, :],
                                g3[:, blk:blk + 2, c0:c0 + cw],
                                start=(im == 0), stop=(im == 7),
                                perf_mode=DR)
                            im += 1
                    nc.vector.scalar_tensor_tensor(
                        out=out_acc[:, mc * ntok + c0:mc * ntok + c0 + cw],
                        in0=ps[:, :cw], scalar=1.0 / (WSCALE * GSCALE),
                        in1=out_acc[:, mc * ntok + c0:mc * ntok + c0 + cw],
                        op0=ALU.mult, op1=ALU.add)
        rel(m2ps, m2w)
        rel(gp1, gp0)

        # ================= Phase D: dense =================
        sap = P("sap", 1, side="right")
        dw = P("dw", 2, side="right")
        dwb = P("dwb", 1, side="right")
        dt_ = P("dt", 2, side="right")
        d2w = P("d2w", 4, side="right")
        dpsa = P("dpsa", 3, "PSUM")
        wb0 = bcast_row(0, dwb, "wb0")
        sa_s = sap.tile([128, 32 * ntok], BF16)
        for grp in range(4):
            wda = dw.tile([128, 64 * 128], BF16, tag="wd1", name="wda")
            nc.sync.dma_start(
                wda[:], wd1a_d[:, grp * 64 * 128:(grp + 1) * 64 * 128])
            for mcl in range(8):
                mc = grp * 8 + mcl
                for (c0, cw) in cts:
                    psa = dpsa.tile([128, 512], F32, tag="dpsa", name="dpsa")
                    for kc in range(DC):
                        nc.tensor.matmul(
                            psa[:, :cw],
                            wda[:, (mcl * 8 + kc) * 128:
                                (mcl * 8 + kc + 1) * 128],
                            x_s[:, kc * nt + HALO + c0:
                                kc * nt + HALO + c0 + cw],
                            start=(kc == 0), stop=(kc == DC - 1))
                    nc.scalar.activation(
                        sa_s[:, mc * ntok + c0:mc * ntok + c0 + cw],
                        psa[:, :cw], AF.Silu, bias=b_d1a[:, mc:mc + 1])
        rel(dpsa)
        dpsb = P("dpsb", 3, "PSUM")

        # prefetch the first dense-fc2 slabs during the b-pass
        wd2_tiles = []

        def fetch_wd2(idx):
            mc, h = idx // 2, idx % 2
            t = d2w.tile([128, 16 * 128], BF16, tag="wd2", name="wd2")
            nc.sync.dma_start(
                t[:], wd2_d[:, (h * 8 + mc) * 16 * 128:
                            (h * 8 + mc + 1) * 16 * 128])
            wd2_tiles.append(t)

        for i in range(4):
            fetch_wd2(i)

        for grp in range(4):
            wdb = dw.tile([128, 64 * 128], BF16, tag="wd1", name="wdb")
            nc.sync.dma_start(
                wdb[:], wd1b_d[:, grp * 64 * 128:(grp + 1) * 64 * 128])
            for mcl in range(8):
                mc = grp * 8 + mcl
                for (c0, cw) in cts:
                    psb = dpsb.tile([128, 512], F32, tag="dpsb", name="dpsb")
                    for kc in range(DC):
                        nc.tensor.matmul(
                            psb[:, :cw],
                            wdb[:, (mcl * 8 + kc) * 128:
                                (mcl * 8 + kc + 1) * 128],
                            x_s[:, kc * nt + HALO + c0:
                                kc * nt + HALO + c0 + cw],
                            start=(kc == 0), stop=(kc == DC - 1))
                    hb2 = dt_.tile([128, 512], BF16, tag="hb2", name="hb2")
                    nc.vector.scalar_tensor_tensor(
                        out=hb2[:, :cw], in0=psb[:, :cw],
                        scalar=b_d1b[:, mc:mc + 1],
                        in1=wb0[:, c0:c0 + cw], op0=ALU.add, op1=ALU.mult)
                    nc.vector.tensor_mul(
                        sa_s[:, mc * ntok + c0:mc * ntok + c0 + cw],
                        sa_s[:, mc * ntok + c0:mc * ntok + c0 + cw],
                        hb2[:, :cw])
        # dense fc2: one 33-matmul chain per (mc, tile) covering both
        # hidden halves plus the collapsed [10,D] bias matmul.
        rel(dpsb)
        d2ps = P("d2ps", 4, "PSUM")
        for mc in range(DC):
            wd2a = wd2_tiles[2 * mc]
            wd2b = wd2_tiles[2 * mc + 1]
            for (c0, cw) in cts:
                ps = d2ps.tile([128, 512], F32, tag="d2psum", name="d2psum")
                nc.tensor.matmul(ps[:, :cw], b10[:, mc * 128:(mc + 1) * 128],
                                 rw10[:, c0:c0 + cw], start=True, stop=False)
                for h, wd2 in ((0, wd2a), (1, wd2b)):
                    for kc in range(16):
                        nc.tensor.matmul(
                            ps[:, :cw], wd2[:, kc * 128:(kc + 1) * 128],
                            sa_s[:, (h * 16 + kc) * ntok + c0:
                                 (h * 16 + kc) * ntok + c0 + cw],
                            start=False, stop=(h == 1 and kc == 15))
                nc.vector.tensor_add(
                    out_acc[:, mc * ntok + c0:mc * ntok + c0 + cw],
                    out_acc[:, mc * ntok + c0:mc * ntok + c0 + cw],
                    ps[:, :cw])
            for (c0, cw) in cts:
                nc.sync.dma_start(
                    out_d[:, mc * ntok + c0:mc * ntok + c0 + cw],
                    out_acc[:, mc * ntok + c0:mc * ntok + c0 + cw])
            if 2 * mc + 5 < 16:
                fetch_wd2(2 * mc + 4)
                fetch_wd2(2 * mc + 5)
        for p in reversed(live):
            p.release()

    nc.compile()
    return nc


# ---------------- host-side packing ----------------

def _pack_mk(WT, kcn, mcn):
    """WT [K, M] -> [128, mcn*kcn*128] with block idx = mc*kcn+kc."""
    return np.ascontiguousarray(
        WT.reshape(kcn, 128, mcn, 128).transpose(1, 2, 0, 3)
        .reshape(128, mcn * kcn * 128))


def _featmajor(xt, ncols):
    """xt [1024, ncols] -> [128, 8*ncols] (kc-blocks along columns)."""
    return np.ascontiguousarray(
        xt.reshape(DC, 128, ncols).transpose(1, 0, 2).reshape(128, DC * ncols))


def _bias_cols(b, n):
    """b [n*128] -> [128, n] with col i = b[i*128:(i+1)*128]."""
    return np.ascontiguousarray(b.reshape(n, 128).T).astype(np.float32)


def _q8(a, scale):
    return np.clip(np.asarray(a, np.float64) * scale,
                   -240.0, 240.0).astype(E4)


def pack_weights(rW, rb, d1W, d1b, d2W, d2b, sW_in, sb_in, sW_conv, sb_conv,
                 sW_out, sb_out, mW, mb, eW1, eb1, eW2, eb2):
    f32 = np.float32
    w = {}
    R = np.concatenate([rW.T, mW.T], axis=1).astype(f32)      # [1024, 11]
    Rh = R.astype(BF)
    Rl = (R - Rh.astype(f32)).astype(BF)
    w["w_rmh"] = _featmajor(Rh, 11)
    w["w_rml"] = _featmajor(Rl, 11)
    w["rm_bias"] = np.concatenate([rb, mb])[:, None].astype(f32)
    w["ident11"] = np.eye(11, dtype=f32)
    w["ident"] = np.eye(128, dtype=BF)
    # selector blocks: block r broadcasts rw10 row r; rows 2-9 carry the
    # moe g-quantization scale GSCALE/WSCALE.
    sel = np.zeros((10, 10 * 128), f32)
    for r in range(10):
        sel[r, r * 128:(r + 1) * 128] = 1.0 if r < 2 else GSCALE / WSCALE
    w["sel10"] = sel.astype(BF)
    # fused ssm taps: N_k = sW_out @ sW_conv[:,:,k] @ sW_in, packed like
    # the conv layout: dst[p, ((oc*4+k)*8+ic)*128+c] = N_k[oc*128+c, ic*128+p]
    sW_out64 = sW_out.astype(np.float64)
    sW_in64 = sW_in.astype(np.float64)
    Nk = np.stack([(sW_out64 @ sW_conv[:, :, k].astype(np.float64)
                    @ sW_in64).T for k in range(KC_)], axis=0)  # [k, i, o]
    A = Nk.astype(BF)
    w["w_conv"] = np.ascontiguousarray(
        A.reshape(4, 8, 128, 8, 128).transpose(2, 3, 0, 1, 4)
        .reshape(128, 8 * 32 * 128))
    # collapsed branch biases: row0 dense, row1 full ssm bias, rows2-9 moe
    b_ssm = (sW_out64 @ (sW_conv.astype(np.float64).sum(axis=2)
                         @ sb_in.astype(np.float64)
                         + sb_conv.astype(np.float64))
             + sb_out.astype(np.float64)).astype(f32)
    b10 = np.stack([d2b, b_ssm] + [eW2b for eW2b in eb2], axis=0)
    w["b10"] = b10.astype(BF)                                  # [10, 1024]
    # experts fc1 (fp8 x WSCALE): block idx e*64 + (j*2+ab)*8 + kc
    morder = [ab * 4 + j for j in range(4) for ab in range(2)]
    slabs = []
    for e in range(E):
        Te = _q8(eW1[e].T, WSCALE).reshape(8, 128, 8, 128)    # kc,p,mc,c
        Te = Te[:, :, morder, :].transpose(1, 2, 0, 3)        # p,jm,kc,c
        slabs.append(Te.reshape(128, 64 * 128))
    w["w_e1"] = np.ascontiguousarray(np.concatenate(slabs, axis=1))
    eb1a = np.stack([eb1[e, j * 128:(j + 1) * 128]
                     for e in range(E) for j in range(4)], axis=1)
    eb1b = np.stack([eb1[e, 512 + j * 128: 512 + (j + 1) * 128]
                     for e in range(E) for j in range(4)], axis=1)
    w["b_e1a"] = eb1a.astype(f32)
    w["b_e1b"] = (eb1b * WSCALE).astype(f32)
    # e2 (fp8 x WSCALE): col block ((egrp*8+mc)*16 + el*4 + kc), e=egrp*4+el
    T5 = np.stack([_q8(eW2[e].T, WSCALE).reshape(4, 128, 8, 128)
                   for e in range(E)])                        # e,kc,p,mc,c
    T6 = T5.reshape(2, 4, 4, 128, 8, 128)                     # g,el,kc,p,mc,c
    w["w_e2"] = np.ascontiguousarray(
        T6.transpose(3, 0, 4, 1, 2, 5).reshape(128, E * 32 * 128))
    w["w_d1a"] = _pack_mk(d1W[:HD].T.astype(BF), 8, 32)
    w["w_d1b"] = _pack_mk(d1W[HD:].T.astype(BF), 8, 32)
    w["b_d1a"] = _bias_cols(d1b[:HD], 32)
    w["b_d1b"] = _bias_cols(d1b[HD:], 32)
    # d2: block idx = h*128 + mc*16 + kcl, kg = h*16+kcl
    T4 = d2W.T.astype(BF).reshape(2, 16, 128, 8, 128)         # h,kcl,p,mc,c
    w["w_d2"] = np.ascontiguousarray(
        T4.transpose(2, 0, 3, 1, 4).reshape(128, 256 * 128))
    return w


def make_in_maps(x, weights, ntok=TOK, ncores=NCORE):
    """x [B,T,D] fp32 -> list of per-core in_maps."""
    xt = np.asarray(x, np.float32).reshape(-1, D).T           # [D, tokens]
    in_maps = []
    for c in range(ncores):
        lo = c * ntok
        xc = xt[:, lo:lo + ntok]
        halo = np.zeros((D, HALO), np.float32)
        if lo >= HALO and lo % T != 0:   # conv is causal per batch element
            halo = xt[:, lo - HALO:lo]
        xch = np.concatenate([halo, xc], axis=1)              # [D, nt]
        m = dict(weights)
        xh = xc.astype(BF)
        m["xl_s"] = _featmajor((xc - xh.astype(np.float32)).astype(BF), ntok)
        m["x_s"] = _featmajor(xch.astype(BF), ntok + HALO)
        m["x8_s"] = _featmajor(np.clip(xc, -240.0, 240.0), ntok).astype(E4)
        in_maps.append(m)
    return in_maps


def assemble_output(results, ntok=TOK, ncores=NCORE):
    cols = []
    for c in range(ncores):
        o = results[c]["outT"]                                # [128, 8*ntok]
        cols.append(o.reshape(128, DC, ntok).transpose(1, 0, 2)
                    .reshape(D, ntok))
    full = np.concatenate(cols, axis=1)                       # [D, tokens]
    return np.ascontiguousarray(full.T).reshape(B, T, D).astype(np.float32)


_CACHED = {}


def kernel(**inputs):
    x = np.asarray(inputs["x"], np.float32)
    names = ["rW", "rb", "d1W", "d1b", "d2W", "d2b", "sW_in", "sb_in",
             "sW_conv", "sb_conv", "sW_out", "sb_out", "mW", "mb",
             "eW1", "eb1", "eW2", "eb2"]
    wargs = [np.asarray(inputs[n], np.float32) for n in names]
    if "nc" not in _CACHED:
        _CACHED["nc"] = build_program(TOK)
    nc = _CACHED["nc"]
    weights = pack_weights(*wargs)
    in_maps = make_in_maps(x, weights)
    res = bass_utils.run_bass_kernel_spmd(
        nc, in_maps, core_ids=list(range(NCORE)))
    return assemble_output(res.results)


# revision 18
# speedup vs baseline: 1.5918x; 1.0132x over previous
"""Trainium2 Bass kernel for nn_EvolutionBlock (moe_routing).

Strategy: data-parallel over the 8192 tokens across 8 NeuronCores
(1024 tokens/core + 3-token halo for the causal conv). Weights are
replicated per core and pre-packed on the host into the exact
[128, cols] SBUF layouts so every DMA is a contiguous slab.

On-chip everything is feature-major ([feature, token]) so matmuls are
out[f_chunk, tok] = lhsT.T @ rhs with lhsT = weight tile [din, dout]
and rhs = activation [din, tok]. Router/top-2 runs token-major in fp32
(selection must match the fp32 reference argmax), gets transposed via
the PE, and the per-token branch weights are broadcast across
partitions with one-hot-row selector matmuls against the [10, ntok]
weight matrix.

Speedups over the bf16 baseline:
 - The SSM branch is linear, so the in-proj, causal conv and out-proj
   collapse into 4 host-precomputed tap matrices
   N_k = sW_out @ sW_conv[:,:,k] @ sW_in; the conv phase consumes x
   directly and its PSUM result (scaled by the branch weight) IS the
   ssm contribution.
 - The whole MoE branch runs in fp8e4m3 with DoubleRow (double-pumped)
   matmuls: expert weights are host-quantized at x64 scale, x at x1,
   and the swiglu output is re-quantized to fp8 at x16; the scales are
   folded into the activation scale, the selector value (0.25)
   and the final 1/1024 accumulate. l2 error ~0.009 (gate 2e-2).
 - Dense swiglu uses the Silu activation directly.
 - Phases are ordered R, C, fc1(all experts), fc2(all), D1a, D1b, D2
   so no PE chain ever waits on its own phase's drain.
Branch combine weights are folded into the fc2 inputs so each branch's
final matmul accumulates the pre-scaled contribution; all branch
biases collapse into one [10, D] bias matmul against router-weight
rows, appended to the dense-fc2 PSUM chains.
"""

import numpy as np
import ml_dtypes

import concourse.bass as bass
import concourse.tile as tile
from concourse import bacc, mybir
from concourse import bass_utils

F32 = mybir.dt.float32
BF16 = mybir.dt.bfloat16
FP8 = mybir.dt.float8e4
AF = mybir.ActivationFunctionType
ALU = mybir.AluOpType
AX = mybir.AxisListType
DR = mybir.MatmulPerfMode.DoubleRow
BF = ml_dtypes.bfloat16
E4 = ml_dtypes.float8_e4m3

# Problem constants
B, T, D = 4, 2048, 1024
HD = 4096          # dense hidden (fc1 out = 2*HD)
S, KC_ = 1024, 4   # ssm state, conv kernel
E, HE = 8, 512     # experts, expert hidden (swiglu)
NCORE = 8
TOKENS = B * T
TOK = TOKENS // NCORE   # tokens per core
HALO = 3
DC = D // 128           # 8 d-chunks

WSCALE = 64.0      # expert weight quantization scale
GSCALE = 16.0      # expert swiglu-output quantization scale


def _coltiles(n, w=512):
    out = []
    c = 0
    while c < n:
        out.append((c, min(w, n - c)))
        c += w
    return out


def build_program(ntok=TOK):
    """Build + compile the Bass program for `ntok` tokens per core."""
    nt = ntok + HALO
    nc = bacc.Bacc("TRN2", target_bir_lowering=False, debug=False,
                   num_devices=NCORE)

    def din(name, shape, dt):
        return nc.dram_tensor(name, list(shape), dt, kind="ExternalInput").ap()

    xl_d = din("xl_s", [128, DC * ntok], BF16)
    xs_d = din("x_s", [128, DC * nt], BF16)
    x8_d = din("x8_s", [128, DC * ntok], FP8)
    wrmh_d = din("w_rmh", [128, DC * 11], BF16)
    wrml_d = din("w_rml", [128, DC * 11], BF16)
    rmb_d = din("rm_bias", [11, 1], F32)
    id11_d = din("ident11", [11, 11], F32)
    ident_d = din("ident", [128, 128], BF16)
    wconv_d = din("w_conv", [128, 8 * 32 * 128], BF16)
    b10_d = din("b10", [10, 1024], BF16)
    we1_d = din("w_e1", [128, E * 64 * 128], FP8)
    be1a_d = din("b_e1a", [128, 32], F32)
    be1b_d = din("b_e1b", [128, 32], F32)
    we2_d = din("w_e2", [128, E * 32 * 128], FP8)
    wd1a_d = din("w_d1a", [128, 256 * 128], BF16)
    wd1b_d = din("w_d1b", [128, 256 * 128], BF16)
    bd1a_d = din("b_d1a", [128, 32], F32)
    bd1b_d = din("b_d1b", [128, 32], F32)
    wd2_d = din("w_d2", [128, 256 * 128], BF16)

    out_d = nc.dram_tensor("outT", [128, DC * ntok], F32,
                           kind="ExternalOutput").ap()

    cts = _coltiles(ntok)
    nchunk = ntok // 128

    with tile.TileContext(nc) as tc:
        live = []

        def P(name, bufs, space="SBUF", side="left"):
            p = tc.alloc_tile_pool(name=name, bufs=bufs, space=space,
                                   side=side)
            live.append(p)
            return p

        def rel(*ps):
            for p in ps:
                live.remove(p)
                p.release()

        constp = P("constp", 1)
        xp = P("xp", 1)
        x8p = P("x8p", 1)

        # Router pools first so the first-needed DMAs issue first.
        rxp = P("rxp", 1, side="right")
        rp = P("rp", nchunk, side="right")
        rps = P("rps", 1, "PSUM", side="right")

        # x/xl arrive in column halves so the first router chain can
        # start after ~1/4 of the activation DMA bytes; xl comes after
        # both x halves because the xh-only logit terms run first.
        x_s = xp.tile([128, DC * nt], BF16)
        xl_s = rxp.tile([128, DC * ntok], BF16)
        xs3s = x_s.rearrange("p (k n) -> p k n", n=nt)
        xs3d = xs_d.rearrange("p (k n) -> p k n", n=nt)
        xl3s = xl_s.rearrange("p (k n) -> p k n", n=ntok)
        xl3d = xl_d.rearrange("p (k n) -> p k n", n=ntok)
        wrmh = rxp.tile([128, DC * 11], BF16)
        nc.sync.dma_start(wrmh[:], wrmh_d[:])
        wrml = rxp.tile([128, DC * 11], BF16)
        nc.sync.dma_start(wrml[:], wrml_d[:])
        nc.sync.dma_start(xs3s[:, :, 0:516], xs3d[:, :, 0:516])
        nc.sync.dma_start(xs3s[:, :, 516:nt], xs3d[:, :, 516:nt])
        nc.sync.dma_start(xl3s[:, :, 0:512], xl3d[:, :, 0:512])
        nc.sync.dma_start(xl3s[:, :, 512:ntok], xl3d[:, :, 512:ntok])
        x8 = x8p.tile([128, DC * ntok], FP8)
        nc.sync.dma_start(x8[:], x8_d[:])
        x83 = x8.rearrange("p (b n) -> p b n", n=ntok)

        ident = constp.tile([128, 128], BF16)
        nc.sync.dma_start(ident[:], ident_d[:])
        rm_bias = constp.tile([11, 1], F32)
        nc.sync.dma_start(rm_bias[:], rmb_d[:])
        ident11 = constp.tile([11, 11], F32)
        nc.sync.dma_start(ident11[:], id11_d[:])
        b10 = constp.tile([10, 1024], BF16)
        nc.sync.dma_start(b10[:], b10_d[:])
        b_e1a = constp.tile([128, 32], F32)
        nc.sync.dma_start(b_e1a[:], be1a_d[:])
        b_e1b = constp.tile([128, 32], F32)
        nc.sync.dma_start(b_e1b[:], be1b_d[:])
        b_d1a = constp.tile([128, 32], F32)
        nc.sync.dma_start(b_d1a[:], bd1a_d[:])
        b_d1b = constp.tile([128, 32], F32)
        nc.sync.dma_start(b_d1b[:], bd1b_d[:])
        rw10 = constp.tile([10, ntok], BF16)
        rwrows = [constp.tile([1, ntok], BF16, tag=f"rwrow{r}",
                              name=f"rwrow{r}") for r in range(10)]
        out_acc = constp.tile([128, DC * ntok], F32)

        def bcast_row(r, pool, tag):
            """[128, ntok] bf16 broadcast of rw10 row r (GpSimd)."""
            wbt = pool.tile([128, ntok], BF16, tag=tag, name=tag)
            nc.gpsimd.partition_broadcast(wbt[:], rwrows[r][0:1, :])
            return wbt

        # ================= Phase R: routers (stage-major) ==========
        rsbs, e3s, tm10s = [], [], []
        # stage 1: exact logits feature-major (3-term bf16 hi/lo); the
        # two xh terms run as a first sub-chain for both tiles so the
        # PE only needs x_s, then the xl sub-chains accumulate on top.
        lg = rxp.tile([11, ntok], F32, tag="lg", name="lg")
        rpss = []
        for (c0, cw) in cts:
            ps = rps.tile([11, 512], F32, tag="ps", name="ps", bufs=2)
            im = 0
            for wrm_t in (wrmh, wrml):
                for kc in range(DC):
                    nc.tensor.matmul(
                        ps[:, :cw], wrm_t[:, kc * 11:(kc + 1) * 11],
                        x_s[:, kc * nt + HALO + c0:kc * nt + HALO + c0 + cw],
                        start=(im == 0), stop=False)
                    im += 1
            rpss.append(ps)
        for (c0, cw), ps in zip(cts, rpss):
            for kc in range(DC):
                nc.tensor.matmul(
                    ps[:, :cw], wrmh[:, kc * 11:(kc + 1) * 11],
                    xl_s[:, kc * ntok + c0:kc * ntok + c0 + cw],
                    start=False, stop=(kc == DC - 1))
            nc.scalar.activation(lg[:, c0:c0 + cw], ps[:, :cw], AF.Identity,
                                 bias=rm_bias[:, 0:1])
        for tcn in range(nchunk):
            pst2 = rps.tile([128, 11], F32, tag="pst2", name="pst2")
            nc.tensor.transpose(pst2[:],
                                lg[:, tcn * 128:(tcn + 1) * 128], ident11[:])
            rsb = rp.tile([128, 11], F32, tag="rsb", name="rsb")
            nc.vector.tensor_copy(rsb[:], pst2[:])
            e3 = rp.tile([128, 3], F32, tag="e3", name="e3")
            nc.scalar.activation(e3[:], rsb[:, 0:3], AF.Exp)
            rsbs.append(rsb)
            e3s.append(e3)
        # stage 2: top-2 + branch weights
        for tcn in range(nchunk):
            rsb, e3 = rsbs[tcn], e3s[tcn]
            s3 = rp.tile([128, 1], F32, tag="s3", name="s3")
            nc.vector.reduce_sum(s3[:], e3[:], axis=AX.X)
            r3 = rp.tile([128, 1], F32, tag="r3", name="r3")
            nc.vector.reciprocal(r3[:], s3[:])
            tm10 = rp.tile([128, 10], BF16, tag="tm10", name="tm10")
            nc.vector.tensor_scalar(out=tm10[:, 0:2], in0=e3[:, 0:2],
                                    scalar1=r3[:], scalar2=None, op0=ALU.mult)
            bw2 = rp.tile([128, 1], F32, tag="bw2", name="bw2")
            nc.vector.tensor_scalar(out=bw2[:], in0=e3[:, 2:3], scalar1=r3[:],
                                    scalar2=None, op0=ALU.mult)
            L = rsb[:, 3:11]
            m1 = rp.tile([128, 1], F32, tag="m1", name="m1")
            nc.vector.reduce_max(m1[:], L, axis=AX.X)
            mask1 = rp.tile([128, 8], F32, tag="mask1", name="mask1")
            nc.vector.tensor_scalar(out=mask1[:], in0=L, scalar1=m1[:],
                                    scalar2=None, op0=ALU.is_equal)
            L2 = rp.tile([128, 8], F32, tag="L2", name="L2")
            nc.vector.scalar_tensor_tensor(out=L2[:], in0=mask1[:],
                                           scalar=-1e9, in1=L,
                                           op0=ALU.mult, op1=ALU.add)
            m2 = rp.tile([128, 1], F32, tag="m2", name="m2")
            nc.vector.reduce_max(m2[:], L2[:], axis=AX.X)
            mask2 = rp.tile([128, 8], F32, tag="mask2", name="mask2")
            nc.vector.tensor_scalar(out=mask2[:], in0=L2[:], scalar1=m2[:],
                                    scalar2=None, op0=ALU.is_equal)
            dv = rp.tile([128, 1], F32, tag="dv", name="dv")
            nc.vector.tensor_sub(dv[:], m1[:], m2[:])
            w1 = rp.tile([128, 1], F32, tag="w1", name="w1")
            nc.scalar.activation(w1[:], dv[:], AF.Sigmoid)
            u1 = rp.tile([128, 1], F32, tag="u1", name="u1")
            nc.vector.tensor_mul(u1[:], w1[:], bw2[:])
            u2 = rp.tile([128, 1], F32, tag="u2", name="u2")
            nc.vector.tensor_sub(u2[:], bw2[:], u1[:])
            c2t = rp.tile([128, 8], F32, tag="c2t", name="c2t")
            nc.vector.tensor_scalar(out=c2t[:], in0=mask2[:], scalar1=u2[:],
                                    scalar2=None, op0=ALU.mult)
            nc.vector.scalar_tensor_tensor(out=tm10[:, 2:10], in0=mask1[:],
                                           scalar=u1[:], in1=c2t[:],
                                           op0=ALU.mult, op1=ALU.add)
            tm10s.append(tm10)
        # stage 3: transpose back to the [10, ntok] weight matrix, then
        # stage each row on partition 0 for the GpSimd broadcasts.
        for tcn in range(nchunk):
            pst = rps.tile([10, 128], BF16, tag="pst2", name="pst")
            nc.tensor.transpose(pst[:], tm10s[tcn][:], ident[:])
            nc.vector.tensor_copy(rw10[:, tcn * 128:(tcn + 1) * 128], pst[:])
        for r in range(10):
            nc.sync.dma_start(rwrows[r][0:1, :], rw10[r:r + 1, :])
        rel(rps, rp, rxp)

        # ====== Phase C: fused ssm (conv of tap matrices), inits out_acc
        cwp = P("cwp", 2, side="right")
        cwt = P("cwt", 1, side="right")
        cps = P("cps", 3, "PSUM")
        wb1 = bcast_row(1, cwt, "wb1")
        for oc in range(DC):
            wcv = cwp.tile([128, 32 * 128], BF16, tag="wcv", name="wcv")
            nc.sync.dma_start(
                wcv[:], wconv_d[:, oc * 32 * 128:(oc + 1) * 32 * 128])
            for (c0, cw) in cts:
                ps = cps.tile([128, 512], F32, tag="cpsum", name="cpsum")
                first = True
                for k in range(KC_):
                    for ic in range(DC):
                        nc.tensor.matmul(
                            ps[:, :cw],
                            wcv[:, (k * 8 + ic) * 128:(k * 8 + ic + 1) * 128],
                            x_s[:, ic * nt + c0 + k:ic * nt + c0 + k + cw],
                            start=first,
                            stop=(k == KC_ - 1 and ic == DC - 1))
                        first = False
                nc.vector.tensor_mul(
                    out_acc[:, oc * ntok + c0:oc * ntok + c0 + cw],
                    ps[:, :cw], wb1[:, c0:c0 + cw])
        rel(cwt, cwp, cps)

        # ================= Phase M: MoE in fp8 =====================
        gp0 = P("gp0", 1, side="right")
        gp1 = P("gp1", 1, side="right")
        m1w = P("m1w", 2, side="right")
        m1t = P("m1t", 2, side="right")
        m1wb = P("m1wb", 2, side="right")
        m1ps = P("m1ps", 2, "PSUM")
        g_s0 = gp0.tile([128, 16 * ntok], FP8, name="g_s0")
        g_s1 = gp1.tile([128, 16 * ntok], FP8, name="g_s1")

        for e in range(E):
            g_s = g_s0 if e < 4 else g_s1
            el = e % 4
            wbm = bcast_row(2 + e, m1wb, "wbm")
            for j in range(4):
                if j % 2 == 0:
                    we1 = m1w.tile([128, 32 * 128], FP8, tag="we1",
                                   name="we1")
                    nc.sync.dma_start(
                        we1[:],
                        we1_d[:, (e * 2 + j // 2) * 32 * 128:
                              (e * 2 + j // 2 + 1) * 32 * 128])
                    w13 = we1.rearrange("p (b f) -> p b f", f=128)
                bcol = e * 4 + j
                for (c0, cw) in cts:
                    psa = m1ps.tile([128, 512], F32, tag="psa", name="psa")
                    psb = m1ps.tile([128, 512], F32, tag="psb", name="psb")
                    for ab, pst_ in ((0, psa), (1, psb)):
                        bi = ((j % 2) * 2 + ab) * 8
                        for t4 in range(4):
                            nc.tensor.matmul(
                                pst_[:, :cw],
                                w13[:, bi + 2 * t4:bi + 2 * t4 + 2, :],
                                x83[:, 2 * t4:2 * t4 + 2, c0:c0 + cw],
                                start=(t4 == 0), stop=(t4 == 3),
                                perf_mode=DR)
                    sg = m1t.tile([128, 512], BF16, tag="sg", name="sg")
                    nc.scalar.activation(
                        sg[:, :cw], psa[:, :cw], AF.Silu,
                        bias=b_e1a[:, bcol:bcol + 1], scale=1.0 / WSCALE)
                    sa2 = m1t.tile([128, 512], BF16, tag="sa2", name="sa2")
                    nc.vector.scalar_tensor_tensor(
                        out=sa2[:, :cw], in0=sg[:, :cw],
                        scalar=GSCALE / WSCALE, in1=wbm[:, c0:c0 + cw],
                        op0=ALU.mult, op1=ALU.mult)
                    nc.vector.scalar_tensor_tensor(
                        out=g_s[:, (el * 4 + j) * ntok + c0:
                                (el * 4 + j) * ntok + c0 + cw],
                        in0=psb[:, :cw],
                        scalar=b_e1b[:, bcol:bcol + 1],
                        in1=sa2[:, :cw], op0=ALU.add, op1=ALU.mult)
        rel(m1ps, m1wb, m1t, m1w)

        m2w = P("m2w", 3)
        m2ps = P("m2ps", 3, "PSUM", side="right")
        g30 = g_s0.rearrange("p (b n) -> p b n", n=ntok)
        g31 = g_s1.rearrange("p (b n) -> p b n", n=ntok)
        for egrp in range(2):
            g3 = g30 if egrp == 0 else g31
            for mc in range(DC):
                we2 = m2w.tile([128, 16 * 128], FP8, tag="we2", name="we2")
                nc.sync.dma_start(
                    we2[:], we2_d[:, (egrp * 8 + mc) * 16 * 128:
                                  (egrp * 8 + mc + 1) * 16 * 128])
                w23 = we2.rearrange("p (b f) -> p b f", f=128)
                for (c0, cw) in cts:
                    ps = m2ps.tile([128, 512], F32, tag="m2psum",
                                   name="m2psum")
                    im = 0
                    for el in range(4):
                        for t2 in range(2):
                            blk = el * 4 + 2 * t2
                            nc.tensor.matmul(
                                ps[:, :cw],
                                w23[:, blk:blk + 2, :],
                                g3[:, blk:blk + 2, c0:c0 + cw],
                                start=(im == 0), stop=(im == 7),
                                perf_mode=DR)
                            im += 1
                    nc.vector.scalar_tensor_tensor(
                        out=out_acc[:, mc * ntok + c0:mc * ntok + c0 + cw],
                        in0=ps[:, :cw], scalar=1.0 / (WSCALE * GSCALE),
                        in1=out_acc[:, mc * ntok + c0:mc * ntok + c0 + cw],
                        op0=ALU.mult, op1=ALU.add)
        rel(m2ps, m2w)
        rel(gp1, gp0)

        # ================= Phase D: dense =================
        sap = P("sap", 1, side="right")
        dw = P("dw", 2, side="right")
        dwb = P("dwb", 1, side="right")
        dt_ = P("dt", 2, side="right")
        d2w = P("d2w", 4, side="right")
        dpsa = P("dpsa", 3, "PSUM")
        wb0 = bcast_row(0, dwb, "wb0")
        sa_s = sap.tile([128, 32 * ntok], BF16)
        for grp in range(4):
            wda = dw.tile([128, 64 * 128], BF16, tag="wd1", name="wda")
            nc.sync.dma_start(
                wda[:], wd1a_d[:, grp * 64 * 128:(grp + 1) * 64 * 128])
            for mcl in range(8):
                mc = grp * 8 + mcl
                for (c0, cw) in cts:
                    psa = dpsa.tile([128, 512], F32, tag="dpsa", name="dpsa")
                    for kc in range(DC):
                        nc.tensor.matmul(
                            psa[:, :cw],
                            wda[:, (mcl * 8 + kc) * 128:
                                (mcl * 8 + kc + 1) * 128],
                            x_s[:, kc * nt + HALO + c0:
                                kc * nt + HALO + c0 + cw],
                            start=(kc == 0), stop=(kc == DC - 1))
                    nc.scalar.activation(
                        sa_s[:, mc * ntok + c0:mc * ntok + c0 + cw],
                        psa[:, :cw], AF.Silu, bias=b_d1a[:, mc:mc + 1])
        rel(dpsa)
        dpsb = P("dpsb", 3, "PSUM")

        # prefetch the first dense-fc2 slabs during the b-pass
        wd2_tiles = []

        def fetch_wd2(idx):
            mc, h = idx // 2, idx % 2
            t = d2w.tile([128, 16 * 128], BF16, tag="wd2", name="wd2")
            nc.sync.dma_start(
                t[:], wd2_d[:, (h * 8 + mc) * 16 * 128:
                            (h * 8 + mc + 1) * 16 * 128])
            wd2_tiles.append(t)

        for i in range(4):
            fetch_wd2(i)

        for grp in range(4):
            wdb = dw.tile([128, 64 * 128], BF16, tag="wd1", name="wdb")
            nc.sync.dma_start(
                wdb[:], wd1b_d[:, grp * 64 * 128:(grp + 1) * 64 * 128])
            for mcl in range(8):
                mc = grp * 8 + mcl
                for (c0, cw) in cts:
                    psb = dpsb.tile([128, 512], F32, tag="dpsb", name="dpsb")
                    for kc in range(DC):
                        nc.tensor.matmul(
                            psb[:, :cw],
                            wdb[:, (mcl * 8 + kc) * 128:
                                (mcl * 8 + kc + 1) * 128],
                            x_s[:, kc * nt + HALO + c0:
                                kc * nt + HALO + c0 + cw],
                            start=(kc == 0), stop=(kc == DC - 1))
                    hb2 = dt_.tile([128, 512], BF16, tag="hb2", name="hb2")
                    nc.vector.scalar_tensor_tensor(
                        out=hb2[:, :cw], in0=psb[:, :cw],
                        scalar=b_d1b[:, mc:mc + 1],
                        in1=wb0[:, c0:c0 + cw], op0=ALU.add, op1=ALU.mult)
                    nc.vector.tensor_mul(
                        sa_s[:, mc * ntok + c0:mc * ntok + c0 + cw],
                        sa_s[:, mc * ntok + c0:mc * ntok + c0 + cw],
                        hb2[:, :cw])
        # dense fc2: one 33-matmul chain per (mc, tile) covering both
        # hidden halves plus the collapsed [10,D] bias matmul.
        rel(dpsb)
        d2ps = P("d2ps", 4, "PSUM")
        for mc in range(DC):
            wd2a = wd2_tiles[2 * mc]
            wd2b = wd2_tiles[2 * mc + 1]
            for (c0, cw) in cts:
                ps = d2ps.tile([128, 512], F32, tag="d2psum", name="d2psum")
                nc.tensor.matmul(ps[:, :cw], b10[:, mc * 128:(mc + 1) * 128],
                                 rw10[:, c0:c0 + cw], start=True, stop=False)
                for h, wd2 in ((0, wd2a), (1, wd2b)):
                    for kc in range(16):
                        nc.tensor.matmul(
                            ps[:, :cw], wd2[:, kc * 128:(kc + 1) * 128],
                            sa_s[:, (h * 16 + kc) * ntok + c0:
                                 (h * 16 + kc) * ntok + c0 + cw],
                            start=False, stop=(h == 1 and kc == 15))
                nc.vector.tensor_add(
                    out_acc[:, mc * ntok + c0:mc * ntok + c0 + cw],
                    out_acc[:, mc * ntok + c0:mc * ntok + c0 + cw],
                    ps[:, :cw])
            for (c0, cw) in cts:
                nc.sync.dma_start(
                    out_d[:, mc * ntok + c0:mc * ntok + c0 + cw],
                    out_acc[:, mc * ntok + c0:mc * ntok + c0 + cw])
            if 2 * mc + 5 < 16:
                fetch_wd2(2 * mc + 4)
                fetch_wd2(2 * mc + 5)
        for p in reversed(live):
            p.release()

    nc.compile()
    return nc


# ---------------- host-side packing ----------------

def _pack_mk(WT, kcn, mcn):
    """WT [K, M] -> [128, mcn*kcn*128] with block idx = mc*kcn+kc."""
    return np.ascontiguousarray(
        WT.reshape(kcn, 128, mcn, 128).transpose(1, 2, 0, 3)
        .reshape(128, mcn * kcn * 128))


def _featmajor(xt, ncols):
    """xt [1024, ncols] -> [128, 8*ncols] (kc-blocks along columns)."""
    return np.ascontiguousarray(
        xt.reshape(DC, 128, ncols).transpose(1, 0, 2).reshape(128, DC * ncols))


def _bias_cols(b, n):
    """b [n*128] -> [128, n] with col i = b[i*128:(i+1)*128]."""
    return np.ascontiguousarray(b.reshape(n, 128).T).astype(np.float32)


def _q8(a, scale):
    return np.clip(np.asarray(a, np.float64) * scale,
                   -240.0, 240.0).astype(E4)


def pack_weights(rW, rb, d1W, d1b, d2W, d2b, sW_in, sb_in, sW_conv, sb_conv,
                 sW_out, sb_out, mW, mb, eW1, eb1, eW2, eb2):
    f32 = np.float32
    w = {}
    R = np.concatenate([rW.T, mW.T], axis=1).astype(f32)      # [1024, 11]
    Rh = R.astype(BF)
    Rl = (R - Rh.astype(f32)).astype(BF)
    w["w_rmh"] = _featmajor(Rh, 11)
    w["w_rml"] = _featmajor(Rl, 11)
    w["rm_bias"] = np.concatenate([rb, mb])[:, None].astype(f32)
    w["ident11"] = np.eye(11, dtype=f32)
    w["ident"] = np.eye(128, dtype=BF)
    # fused ssm taps: N_k = sW_out @ sW_conv[:,:,k] @ sW_in, packed like
    # the conv layout: dst[p, ((oc*4+k)*8+ic)*128+c] = N_k[oc*128+c, ic*128+p]
    sW_out64 = sW_out.astype(np.float64)
    sW_in64 = sW_in.astype(np.float64)
    Nk = np.stack([(sW_out64 @ sW_conv[:, :, k].astype(np.float64)
                    @ sW_in64).T for k in range(KC_)], axis=0)  # [k, i, o]
    A = Nk.astype(BF)
    w["w_conv"] = np.ascontiguousarray(
        A.reshape(4, 8, 128, 8, 128).transpose(2, 3, 0, 1, 4)
        .reshape(128, 8 * 32 * 128))
    # collapsed branch biases: row0 dense, row1 full ssm bias, rows2-9 moe
    b_ssm = (sW_out64 @ (sW_conv.astype(np.float64).sum(axis=2)
                         @ sb_in.astype(np.float64)
                         + sb_conv.astype(np.float64))
             + sb_out.astype(np.float64)).astype(f32)
    b10 = np.stack([d2b, b_ssm] + [eW2b for eW2b in eb2], axis=0)
    w["b10"] = b10.astype(BF)                                  # [10, 1024]
    # experts fc1 (fp8 x WSCALE): block idx e*64 + (j*2+ab)*8 + kc
    morder = [ab * 4 + j for j in range(4) for ab in range(2)]
    slabs = []
    for e in range(E):
        Te = _q8(eW1[e].T, WSCALE).reshape(8, 128, 8, 128)    # kc,p,mc,c
        Te = Te[:, :, morder, :].transpose(1, 2, 0, 3)        # p,jm,kc,c
        slabs.append(Te.reshape(128, 64 * 128))
    w["w_e1"] = np.ascontiguousarray(np.concatenate(slabs, axis=1))
    eb1a = np.stack([eb1[e, j * 128:(j + 1) * 128]
                     for e in range(E) for j in range(4)], axis=1)
    eb1b = np.stack([eb1[e, 512 + j * 128: 512 + (j + 1) * 128]
                     for e in range(E) for j in range(4)], axis=1)
    w["b_e1a"] = eb1a.astype(f32)
    w["b_e1b"] = (eb1b * WSCALE).astype(f32)
    # e2 (fp8 x WSCALE): col block ((egrp*8+mc)*16 + el*4 + kc), e=egrp*4+el
    T5 = np.stack([_q8(eW2[e].T, WSCALE).reshape(4, 128, 8, 128)
                   for e in range(E)])                        # e,kc,p,mc,c
    T6 = T5.reshape(2, 4, 4, 128, 8, 128)                     # g,el,kc,p,mc,c
    w["w_e2"] = np.ascontiguousarray(
        T6.transpose(3, 0, 4, 1, 2, 5).reshape(128, E * 32 * 128))
    w["w_d1a"] = _pack_mk(d1W[:HD].T.astype(BF), 8, 32)
    w["w_d1b"] = _pack_mk(d1W[HD:].T.astype(BF), 8, 32)
    w["b_d1a"] = _bias_cols(d1b[:HD], 32)
    w["b_d1b"] = _bias_cols(d1b[HD:], 32)
    # d2: block idx = h*128 + mc*16 + kcl, kg = h*16+kcl
    T4 = d2W.T.astype(BF).reshape(2, 16, 128, 8, 128)         # h,kcl,p,mc,c
    w["w_d2"] = np.ascontiguousarray(
        T4.transpose(2, 0, 3, 1, 4).reshape(128, 256 * 128))
    return w


def make_in_maps(x, weights, ntok=TOK, ncores=NCORE):
    """x [B,T,D] fp32 -> list of per-core in_maps."""
    xt = np.asarray(x, np.float32).reshape(-1, D).T           # [D, tokens]
    in_maps = []
    for c in range(ncores):
        lo = c * ntok
        xc = xt[:, lo:lo + ntok]
        halo = np.zeros((D, HALO), np.float32)
        if lo >= HALO and lo % T != 0:   # conv is causal per batch element
            halo = xt[:, lo - HALO:lo]
        xch = np.concatenate([halo, xc], axis=1)              # [D, nt]
        m = dict(weights)
        xh = xc.astype(BF)
        m["xl_s"] = _featmajor((xc - xh.astype(np.float32)).astype(BF), ntok)
        m["x_s"] = _featmajor(xch.astype(BF), ntok + HALO)
        m["x8_s"] = _featmajor(np.clip(xc, -240.0, 240.0), ntok).astype(E4)
        in_maps.append(m)
    return in_maps


def assemble_output(results, ntok=TOK, ncores=NCORE):
    cols = []
    for c in range(ncores):
        o = results[c]["outT"]                                # [128, 8*ntok]
        cols.append(o.reshape(128, DC, ntok).transpose(1, 0, 2)
                    .reshape(D, ntok))
    full = np.concatenate(cols, axis=1)                       # [D, tokens]
    return np.ascontiguousarray(full.T).reshape(B, T, D).astype(np.float32)


_CACHED = {}


def kernel(**inputs):
    x = np.asarray(inputs["x"], np.float32)
    names = ["rW", "rb", "d1W", "d1b", "d2W", "d2b", "sW_in", "sb_in",
             "sW_conv", "sb_conv", "sW_out", "sb_out", "mW", "mb",
             "eW1", "eb1", "eW2", "eb2"]
    wargs = [np.asarray(inputs[n], np.float32) for n in names]
    if "nc" not in _CACHED:
        _CACHED["nc"] = build_program(TOK)
    nc = _CACHED["nc"]
    weights = pack_weights(*wargs)
    in_maps = make_in_maps(x, weights)
    res = bass_utils.run_bass_kernel_spmd(
        nc, in_maps, core_ids=list(range(NCORE)))
    return assemble_output(res.results)


# revision 22
# speedup vs baseline: 1.6010x; 1.0058x over previous
"""Trainium2 Bass kernel for nn_EvolutionBlock (moe_routing).

Strategy: data-parallel over the 8192 tokens across 8 NeuronCores
(1024 tokens/core + 3-token halo for the causal conv). Weights are
replicated per core and pre-packed on the host into the exact
[128, cols] SBUF layouts so every DMA is a contiguous slab.

On-chip everything is feature-major ([feature, token]) so matmuls are
out[f_chunk, tok] = lhsT.T @ rhs with lhsT = weight tile [din, dout]
and rhs = activation [din, tok]. Router/top-2 runs token-major in fp32
(selection must match the fp32 reference argmax), gets transposed via
the PE, and the per-token branch weights are broadcast across
partitions with one-hot-row selector matmuls against the [10, ntok]
weight matrix.

Speedups over the bf16 baseline:
 - The SSM branch is linear, so the in-proj, causal conv and out-proj
   collapse into 4 host-precomputed tap matrices
   N_k = sW_out @ sW_conv[:,:,k] @ sW_in; the conv phase consumes x
   directly and its PSUM result (scaled by the branch weight) IS the
   ssm contribution.
 - The whole MoE branch runs in fp8e4m3 with DoubleRow (double-pumped)
   matmuls: expert weights are host-quantized at x64 scale, x at x1,
   and the swiglu output is re-quantized to fp8 at x16; the scales are
   folded into the activation scale, the selector value (0.25)
   and the final 1/1024 accumulate. l2 error ~0.009 (gate 2e-2).
 - Dense swiglu uses the Silu activation directly.
 - Phases are ordered R, C, fc1(all experts), fc2(all), D1a, D1b, D2
   so no PE chain ever waits on its own phase's drain.
Branch combine weights are folded into the fc2 inputs so each branch's
final matmul accumulates the pre-scaled contribution; all branch
biases collapse into one [10, D] bias matmul against router-weight
rows, appended to the dense-fc2 PSUM chains.
"""

import numpy as np
import ml_dtypes

import concourse.bass as bass
import concourse.tile as tile
from concourse import bacc, mybir
from concourse import bass_utils

F32 = mybir.dt.float32
BF16 = mybir.dt.bfloat16
FP8 = mybir.dt.float8e4
AF = mybir.ActivationFunctionType
ALU = mybir.AluOpType
AX = mybir.AxisListType
DR = mybir.MatmulPerfMode.DoubleRow
BF = ml_dtypes.bfloat16
E4 = ml_dtypes.float8_e4m3

# Problem constants
B, T, D = 4, 2048, 1024
HD = 4096          # dense hidden (fc1 out = 2*HD)
S, KC_ = 1024, 4   # ssm state, conv kernel
E, HE = 8, 512     # experts, expert hidden (swiglu)
NCORE = 8
TOKENS = B * T
TOK = TOKENS // NCORE   # tokens per core
HALO = 3
DC = D // 128           # 8 d-chunks

WSCALE = 64.0      # expert weight quantization scale
GSCALE = 16.0      # expert swiglu-output quantization scale


def _coltiles(n, w=512):
    out = []
    c = 0
    while c < n:
        out.append((c, min(w, n - c)))
        c += w
    return out


def build_program(ntok=TOK):
    """Build + compile the Bass program for `ntok` tokens per core."""
    nt = ntok + HALO
    nc = bacc.Bacc("TRN2", target_bir_lowering=False, debug=False,
                   num_devices=NCORE)

    def din(name, shape, dt):
        return nc.dram_tensor(name, list(shape), dt, kind="ExternalInput").ap()

    xl_d = din("xl_s", [128, DC * ntok], BF16)
    xs_d = din("x_s", [128, DC * nt], BF16)
    x8_d = din("x8_s", [128, DC * ntok], FP8)
    wrmh_d = din("w_rmh", [128, DC * 11], BF16)
    wrml_d = din("w_rml", [128, DC * 11], BF16)
    rmb_d = din("rm_bias", [11, 1], F32)
    id11_d = din("ident11", [11, 11], F32)
    ident_d = din("ident", [128, 128], BF16)
    wconv_d = din("w_conv", [128, 8 * 32 * 128], BF16)
    b10_d = din("b10", [10, 1024], BF16)
    we1_d = din("w_e1", [128, E * 64 * 128], FP8)
    be1a_d = din("b_e1a", [128, 32], F32)
    be1b_d = din("b_e1b", [128, 32], F32)
    we2_d = din("w_e2", [128, E * 32 * 128], FP8)
    wd1a_d = din("w_d1a", [128, 256 * 128], BF16)
    wd1b_d = din("w_d1b", [128, 256 * 128], BF16)
    bd1a_d = din("b_d1a", [128, 32], F32)
    bd1b_d = din("b_d1b", [128, 32], F32)
    wd2_d = din("w_d2", [128, 256 * 128], BF16)

    out_d = nc.dram_tensor("outT", [128, DC * ntok], F32,
                           kind="ExternalOutput").ap()

    cts = _coltiles(ntok)
    nchunk = ntok // 128

    with tile.TileContext(nc) as tc:
        live = []

        def P(name, bufs, space="SBUF", side="left"):
            p = tc.alloc_tile_pool(name=name, bufs=bufs, space=space,
                                   side=side)
            live.append(p)
            return p

        def rel(*ps):
            for p in ps:
                live.remove(p)
                p.release()

        constp = P("constp", 1)
        xp = P("xp", 1)
        x8p = P("x8p", 1)

        # Router pools first so the first-needed DMAs issue first.
        rxp = P("rxp", 1, side="right")
        rp = P("rp", nchunk, side="right")
        rps = P("rps", 1, "PSUM", side="right")

        # x/xl arrive in column halves so the first router chain can
        # start after ~1/4 of the activation DMA bytes; xl comes after
        # both x halves because the xh-only logit terms run first.
        x_s = xp.tile([128, DC * nt], BF16)
        xl_s = rxp.tile([128, DC * ntok], BF16)
        xs3s = x_s.rearrange("p (k n) -> p k n", n=nt)
        xs3d = xs_d.rearrange("p (k n) -> p k n", n=nt)
        xl3s = xl_s.rearrange("p (k n) -> p k n", n=ntok)
        xl3d = xl_d.rearrange("p (k n) -> p k n", n=ntok)
        wrmh = rxp.tile([128, DC * 11], BF16)
        nc.sync.dma_start(wrmh[:], wrmh_d[:])
        wrml = rxp.tile([128, DC * 11], BF16)
        nc.sync.dma_start(wrml[:], wrml_d[:])
        nc.sync.dma_start(xs3s[:, :, 0:516], xs3d[:, :, 0:516])
        nc.sync.dma_start(xs3s[:, :, 516:nt], xs3d[:, :, 516:nt])
        nc.sync.dma_start(xl3s[:, :, 0:512], xl3d[:, :, 0:512])
        nc.sync.dma_start(xl3s[:, :, 512:ntok], xl3d[:, :, 512:ntok])
        x8 = x8p.tile([128, DC * ntok], FP8)
        nc.sync.dma_start(x8[:], x8_d[:])
        x83 = x8.rearrange("p (b n) -> p b n", n=ntok)

        ident = constp.tile([128, 128], BF16)
        nc.sync.dma_start(ident[:], ident_d[:])
        rm_bias = constp.tile([11, 1], F32)
        nc.sync.dma_start(rm_bias[:], rmb_d[:])
        ident11 = constp.tile([11, 11], F32)
        nc.sync.dma_start(ident11[:], id11_d[:])
        b10 = constp.tile([10, 1024], BF16)
        nc.sync.dma_start(b10[:], b10_d[:])
        b_e1a = constp.tile([128, 32], F32)
        nc.sync.dma_start(b_e1a[:], be1a_d[:])
        b_e1b = constp.tile([128, 32], F32)
        nc.sync.dma_start(b_e1b[:], be1b_d[:])
        b_d1a = constp.tile([128, 32], F32)
        nc.sync.dma_start(b_d1a[:], bd1a_d[:])
        b_d1b = constp.tile([128, 32], F32)
        nc.sync.dma_start(b_d1b[:], bd1b_d[:])
        rw10 = constp.tile([10, ntok], BF16)
        rwrows = [constp.tile([1, ntok], BF16, tag=f"rwrow{r}",
                              name=f"rwrow{r}") for r in range(10)]
        out_acc = constp.tile([128, DC * ntok], F32)

        def bcast_row(r, pool, tag):
            """[128, ntok] bf16 broadcast of rw10 row r (GpSimd)."""
            wbt = pool.tile([128, ntok], BF16, tag=tag, name=tag)
            nc.gpsimd.partition_broadcast(wbt[:], rwrows[r][0:1, :])
            return wbt

        # ================= Phase R: routers (stage-major) ==========
        rsbs, e3s, tm10s = [], [], []
        # stage 1: exact logits feature-major (3-term bf16 hi/lo); the
        # two xh terms run as a first sub-chain for both tiles so the
        # PE only needs x_s, then the xl sub-chains accumulate on top.
        lg = rxp.tile([11, ntok], F32, tag="lg", name="lg")
        rpss = []
        for (c0, cw) in cts:
            ps = rps.tile([11, 512], F32, tag="ps", name="ps", bufs=2)
            im = 0
            for wrm_t in (wrmh, wrml):
                for kc in range(DC):
                    nc.tensor.matmul(
                        ps[:, :cw], wrm_t[:, kc * 11:(kc + 1) * 11],
                        x_s[:, kc * nt + HALO + c0:kc * nt + HALO + c0 + cw],
                        start=(im == 0), stop=False)
                    im += 1
            rpss.append(ps)
        for (c0, cw), ps in zip(cts, rpss):
            for kc in range(DC):
                nc.tensor.matmul(
                    ps[:, :cw], wrmh[:, kc * 11:(kc + 1) * 11],
                    xl_s[:, kc * ntok + c0:kc * ntok + c0 + cw],
                    start=False, stop=(kc == DC - 1))
            nc.scalar.activation(lg[:, c0:c0 + cw], ps[:, :cw], AF.Identity,
                                 bias=rm_bias[:, 0:1])
        for tcn in range(nchunk):
            pst2 = rps.tile([128, 11], F32, tag="pst2", name="pst2")
            nc.tensor.transpose(pst2[:],
                                lg[:, tcn * 128:(tcn + 1) * 128], ident11[:])
            rsb = rp.tile([128, 11], F32, tag="rsb", name="rsb")
            nc.vector.tensor_copy(rsb[:], pst2[:])
            e3 = rp.tile([128, 3], F32, tag="e3", name="e3")
            nc.scalar.activation(e3[:], rsb[:, 0:3], AF.Exp)
            rsbs.append(rsb)
            e3s.append(e3)
        # stage 2: top-2 + branch weights
        for tcn in range(nchunk):
            rsb, e3 = rsbs[tcn], e3s[tcn]
            s3 = rp.tile([128, 1], F32, tag="s3", name="s3")
            nc.vector.reduce_sum(s3[:], e3[:], axis=AX.X)
            r3 = rp.tile([128, 1], F32, tag="r3", name="r3")
            nc.vector.reciprocal(r3[:], s3[:])
            tm10 = rp.tile([128, 10], BF16, tag="tm10", name="tm10")
            nc.vector.tensor_scalar(out=tm10[:, 0:2], in0=e3[:, 0:2],
                                    scalar1=r3[:], scalar2=None, op0=ALU.mult)
            bw2 = rp.tile([128, 1], F32, tag="bw2", name="bw2")
            nc.vector.tensor_scalar(out=bw2[:], in0=e3[:, 2:3], scalar1=r3[:],
                                    scalar2=None, op0=ALU.mult)
            L = rsb[:, 3:11]
            m1 = rp.tile([128, 1], F32, tag="m1", name="m1")
            nc.vector.reduce_max(m1[:], L, axis=AX.X)
            mask1 = rp.tile([128, 8], F32, tag="mask1", name="mask1")
            nc.vector.tensor_scalar(out=mask1[:], in0=L, scalar1=m1[:],
                                    scalar2=None, op0=ALU.is_equal)
            L2 = rp.tile([128, 8], F32, tag="L2", name="L2")
            nc.vector.scalar_tensor_tensor(out=L2[:], in0=mask1[:],
                                           scalar=-1e9, in1=L,
                                           op0=ALU.mult, op1=ALU.add)
            m2 = rp.tile([128, 1], F32, tag="m2", name="m2")
            nc.vector.reduce_max(m2[:], L2[:], axis=AX.X)
            mask2 = rp.tile([128, 8], F32, tag="mask2", name="mask2")
            nc.vector.tensor_scalar(out=mask2[:], in0=L2[:], scalar1=m2[:],
                                    scalar2=None, op0=ALU.is_equal)
            dv = rp.tile([128, 1], F32, tag="dv", name="dv")
            nc.vector.tensor_sub(dv[:], m1[:], m2[:])
            w1 = rp.tile([128, 1], F32, tag="w1", name="w1")
            nc.scalar.activation(w1[:], dv[:], AF.Sigmoid)
            u1 = rp.tile([128, 1], F32, tag="u1", name="u1")
            nc.vector.tensor_mul(u1[:], w1[:], bw2[:])
            u2 = rp.tile([128, 1], F32, tag="u2", name="u2")
            nc.vector.tensor_sub(u2[:], bw2[:], u1[:])
            c2t = rp.tile([128, 8], F32, tag="c2t", name="c2t")
            nc.vector.tensor_scalar(out=c2t[:], in0=mask2[:], scalar1=u2[:],
                                    scalar2=None, op0=ALU.mult)
            nc.vector.scalar_tensor_tensor(out=tm10[:, 2:10], in0=mask1[:],
                                           scalar=u1[:], in1=c2t[:],
                                           op0=ALU.mult, op1=ALU.add)
            tm10s.append(tm10)
        # stage 3: transpose back to the [10, ntok] weight matrix, then
        # stage each row on partition 0 for the GpSimd broadcasts.
        for tcn in range(nchunk):
            pst = rps.tile([10, 128], BF16, tag="pst2", name="pst")
            nc.tensor.transpose(pst[:], tm10s[tcn][:], ident[:])
            nc.vector.tensor_copy(rw10[:, tcn * 128:(tcn + 1) * 128], pst[:])
        for r in [1, 2, 3, 4, 5, 6, 7, 8, 9, 0]:   # wb1's row first
            nc.sync.dma_start(rwrows[r][0:1, :], rw10[r:r + 1, :])
        rel(rps, rp, rxp)

        # ====== Phase C: fused ssm (conv of tap matrices), inits out_acc
        cwp = P("cwp", 2, side="right")
        cwt = P("cwt", 1, side="right")
        cps = P("cps", 5, "PSUM")
        wb1 = bcast_row(1, cwt, "wb1")
        for oc in range(DC):
            wcv = cwp.tile([128, 32 * 128], BF16, tag="wcv", name="wcv")
            nc.sync.dma_start(
                wcv[:], wconv_d[:, oc * 32 * 128:(oc + 1) * 32 * 128])
            for (c0, cw) in cts:
                ps = cps.tile([128, 512], F32, tag="cpsum", name="cpsum")
                first = True
                for k in range(KC_):
                    for ic in range(DC):
                        nc.tensor.matmul(
                            ps[:, :cw],
                            wcv[:, (k * 8 + ic) * 128:(k * 8 + ic + 1) * 128],
                            x_s[:, ic * nt + c0 + k:ic * nt + c0 + k + cw],
                            start=first,
                            stop=(k == KC_ - 1 and ic == DC - 1))
                        first = False
                nc.vector.tensor_mul(
                    out_acc[:, oc * ntok + c0:oc * ntok + c0 + cw],
                    ps[:, :cw], wb1[:, c0:c0 + cw])
        rel(cwt, cwp, cps)

        # ================= Phase M: MoE in fp8 =====================
        gp0 = P("gp0", 1, side="right")
        gp1 = P("gp1", 1, side="right")
        m1w = P("m1w", 2, side="right")
        m1t = P("m1t", 2, side="right")
        m1wb = P("m1wb", 2, side="right")
        m1ps = P("m1ps", 2, "PSUM")
        g_s0 = gp0.tile([128, 16 * ntok], FP8, name="g_s0")
        g_s1 = gp1.tile([128, 16 * ntok], FP8, name="g_s1")

        for e in range(E):
            g_s = g_s0 if e < 4 else g_s1
            el = e % 4
            wbm = bcast_row(2 + e, m1wb, "wbm")
            for j in range(4):
                if j % 2 == 0:
                    we1 = m1w.tile([128, 32 * 128], FP8, tag="we1",
                                   name="we1")
                    nc.sync.dma_start(
                        we1[:],
                        we1_d[:, (e * 2 + j // 2) * 32 * 128:
                              (e * 2 + j // 2 + 1) * 32 * 128])
                    w13 = we1.rearrange("p (b f) -> p b f", f=128)
                bcol = e * 4 + j
                for (c0, cw) in cts:
                    psa = m1ps.tile([128, 512], F32, tag="psa", name="psa")
                    psb = m1ps.tile([128, 512], F32, tag="psb", name="psb")
                    for ab, pst_ in ((0, psa), (1, psb)):
                        bi = ((j % 2) * 2 + ab) * 8
                        for t4 in range(4):
                            nc.tensor.matmul(
                                pst_[:, :cw],
                                w13[:, bi + 2 * t4:bi + 2 * t4 + 2, :],
                                x83[:, 2 * t4:2 * t4 + 2, c0:c0 + cw],
                                start=(t4 == 0), stop=(t4 == 3),
                                perf_mode=DR)
                    sg = m1t.tile([128, 512], BF16, tag="sg", name="sg")
                    nc.scalar.activation(
                        sg[:, :cw], psa[:, :cw], AF.Silu,
                        bias=b_e1a[:, bcol:bcol + 1], scale=1.0 / WSCALE)
                    sa2 = m1t.tile([128, 512], BF16, tag="sa2", name="sa2")
                    nc.vector.scalar_tensor_tensor(
                        out=sa2[:, :cw], in0=sg[:, :cw],
                        scalar=GSCALE / WSCALE, in1=wbm[:, c0:c0 + cw],
                        op0=ALU.mult, op1=ALU.mult)
                    nc.vector.scalar_tensor_tensor(
                        out=g_s[:, (el * 4 + j) * ntok + c0:
                                (el * 4 + j) * ntok + c0 + cw],
                        in0=psb[:, :cw],
                        scalar=b_e1b[:, bcol:bcol + 1],
                        in1=sa2[:, :cw], op0=ALU.add, op1=ALU.mult)
        rel(m1ps, m1wb, m1t, m1w)

        m2w = P("m2w", 3)
        m2ps = P("m2ps", 3, "PSUM", side="right")
        g30 = g_s0.rearrange("p (b n) -> p b n", n=ntok)
        g31 = g_s1.rearrange("p (b n) -> p b n", n=ntok)
        for egrp in range(2):
            g3 = g30 if egrp == 0 else g31
            for mc in range(DC):
                we2 = m2w.tile([128, 16 * 128], FP8, tag="we2", name="we2")
                nc.sync.dma_start(
                    we2[:], we2_d[:, (egrp * 8 + mc) * 16 * 128:
                                  (egrp * 8 + mc + 1) * 16 * 128])
                w23 = we2.rearrange("p (b f) -> p b f", f=128)
                for (c0, cw) in cts:
                    ps = m2ps.tile([128, 512], F32, tag="m2psum",
                                   name="m2psum")
                    im = 0
                    for el in range(4):
                        for t2 in range(2):
                            blk = el * 4 + 2 * t2
                            nc.tensor.matmul(
                                ps[:, :cw],
                                w23[:, blk:blk + 2, :],
                                g3[:, blk:blk + 2, c0:c0 + cw],
                                start=(im == 0), stop=(im == 7),
                                perf_mode=DR)
                            im += 1
                    nc.vector.scalar_tensor_tensor(
                        out=out_acc[:, mc * ntok + c0:mc * ntok + c0 + cw],
                        in0=ps[:, :cw], scalar=1.0 / (WSCALE * GSCALE),
                        in1=out_acc[:, mc * ntok + c0:mc * ntok + c0 + cw],
                        op0=ALU.mult, op1=ALU.add)
        rel(m2ps, m2w)
        rel(gp1, gp0)

        # ================= Phase D: dense =================
        sap = P("sap", 1, side="right")
        dw = P("dw", 2, side="right")
        dwb = P("dwb", 1, side="right")
        dt_ = P("dt", 2, side="right")
        d2w = P("d2w", 4, side="right")
        dpsa = P("dpsa", 3, "PSUM")
        wb0 = bcast_row(0, dwb, "wb0")
        sa_s = sap.tile([128, 32 * ntok], BF16)
        for grp in range(4):
            wda = dw.tile([128, 64 * 128], BF16, tag="wd1", name="wda")
            nc.sync.dma_start(
                wda[:], wd1a_d[:, grp * 64 * 128:(grp + 1) * 64 * 128])
            for mcl in range(8):
                mc = grp * 8 + mcl
                for (c0, cw) in cts:
                    psa = dpsa.tile([128, 512], F32, tag="dpsa", name="dpsa")
                    for kc in range(DC):
                        nc.tensor.matmul(
                            psa[:, :cw],
                            wda[:, (mcl * 8 + kc) * 128:
                                (mcl * 8 + kc + 1) * 128],
                            x_s[:, kc * nt + HALO + c0:
                                kc * nt + HALO + c0 + cw],
                            start=(kc == 0), stop=(kc == DC - 1))
                    nc.scalar.activation(
                        sa_s[:, mc * ntok + c0:mc * ntok + c0 + cw],
                        psa[:, :cw], AF.Silu, bias=b_d1a[:, mc:mc + 1])
        rel(dpsa)
        dpsb = P("dpsb", 3, "PSUM")

        # prefetch the first dense-fc2 slabs during the b-pass
        wd2_tiles = []

        def fetch_wd2(idx):
            mc, h = idx // 2, idx % 2
            t = d2w.tile([128, 16 * 128], BF16, tag="wd2", name="wd2")
            nc.sync.dma_start(
                t[:], wd2_d[:, (h * 8 + mc) * 16 * 128:
                            (h * 8 + mc + 1) * 16 * 128])
            wd2_tiles.append(t)

        for i in range(4):
            fetch_wd2(i)

        for grp in range(4):
            wdb = dw.tile([128, 64 * 128], BF16, tag="wd1", name="wdb")
            nc.sync.dma_start(
                wdb[:], wd1b_d[:, grp * 64 * 128:(grp + 1) * 64 * 128])
            for mcl in range(8):
                mc = grp * 8 + mcl
                for (c0, cw) in cts:
                    psb = dpsb.tile([128, 512], F32, tag="dpsb", name="dpsb")
                    for kc in range(DC):
                        nc.tensor.matmul(
                            psb[:, :cw],
                            wdb[:, (mcl * 8 + kc) * 128:
                                (mcl * 8 + kc + 1) * 128],
                            x_s[:, kc * nt + HALO + c0:
                                kc * nt + HALO + c0 + cw],
                            start=(kc == 0), stop=(kc == DC - 1))
                    hb2 = dt_.tile([128, 512], BF16, tag="hb2", name="hb2")
                    nc.vector.scalar_tensor_tensor(
                        out=hb2[:, :cw], in0=psb[:, :cw],
                        scalar=b_d1b[:, mc:mc + 1],
                        in1=wb0[:, c0:c0 + cw], op0=ALU.add, op1=ALU.mult)
                    nc.vector.tensor_mul(
                        sa_s[:, mc * ntok + c0:mc * ntok + c0 + cw],
                        sa_s[:, mc * ntok + c0:mc * ntok + c0 + cw],
                        hb2[:, :cw])
        # dense fc2: one 33-matmul chain per (mc, tile) covering both
        # hidden halves plus the collapsed [10,D] bias matmul.
        rel(dpsb)
        d2ps = P("d2ps", 4, "PSUM")
        for mc in range(DC):
            wd2a = wd2_tiles[2 * mc]
            wd2b = wd2_tiles[2 * mc + 1]
            # finer tiles on the last chunk shorten the drain tail
            for (c0, cw) in (cts if mc < DC - 1 else _coltiles(ntok, 256)):
                ps = d2ps.tile([128, 512], F32, tag="d2psum", name="d2psum")
                nc.tensor.matmul(ps[:, :cw], b10[:, mc * 128:(mc + 1) * 128],
                                 rw10[:, c0:c0 + cw], start=True, stop=False)
                for h, wd2 in ((0, wd2a), (1, wd2b)):
                    for kc in range(16):
                        nc.tensor.matmul(
                            ps[:, :cw], wd2[:, kc * 128:(kc + 1) * 128],
                            sa_s[:, (h * 16 + kc) * ntok + c0:
                                 (h * 16 + kc) * ntok + c0 + cw],
                            start=False, stop=(h == 1 and kc == 15))
                nc.vector.tensor_add(
                    out_acc[:, mc * ntok + c0:mc * ntok + c0 + cw],
                    out_acc[:, mc * ntok + c0:mc * ntok + c0 + cw],
                    ps[:, :cw])
            for (c0, cw) in (cts if mc < DC - 1 else _coltiles(ntok, 256)):
                nc.sync.dma_start(
                    out_d[:, mc * ntok + c0:mc * ntok + c0 + cw],
                    out_acc[:, mc * ntok + c0:mc * ntok + c0 + cw])
            if 2 * mc + 5 < 16:
                fetch_wd2(2 * mc + 4)
                fetch_wd2(2 * mc + 5)
        for p in reversed(live):
            p.release()

    nc.compile()
    return nc


# ---------------- host-side packing ----------------

def _pack_mk(WT, kcn, mcn):
    """WT [K, M] -> [128, mcn*kcn*128] with block idx = mc*kcn+kc."""
    return np.ascontiguousarray(
        WT.reshape(kcn, 128, mcn, 128).transpose(1, 2, 0, 3)
        .reshape(128, mcn * kcn * 128))


def _featmajor(xt, ncols):
    """xt [1024, ncols] -> [128, 8*ncols] (kc-blocks along columns)."""
    return np.ascontiguousarray(
        xt.reshape(DC, 128, ncols).transpose(1, 0, 2).reshape(128, DC * ncols))


def _bias_cols(b, n):
    """b [n*128] -> [128, n] with col i = b[i*128:(i+1)*128]."""
    return np.ascontiguousarray(b.reshape(n, 128).T).astype(np.float32)


def _q8(a, scale):
    return np.clip(np.asarray(a, np.float64) * scale,
                   -240.0, 240.0).astype(E4)


def pack_weights(rW, rb, d1W, d1b, d2W, d2b, sW_in, sb_in, sW_conv, sb_conv,
                 sW_out, sb_out, mW, mb, eW1, eb1, eW2, eb2):
    f32 = np.float32
    w = {}
    R = np.concatenate([rW.T, mW.T], axis=1).astype(f32)      # [1024, 11]
    Rh = R.astype(BF)
    Rl = (R - Rh.astype(f32)).astype(BF)
    w["w_rmh"] = _featmajor(Rh, 11)
    w["w_rml"] = _featmajor(Rl, 11)
    w["rm_bias"] = np.concatenate([rb, mb])[:, None].astype(f32)
    w["ident11"] = np.eye(11, dtype=f32)
    w["ident"] = np.eye(128, dtype=BF)
    # fused ssm taps: N_k = sW_out @ sW_conv[:,:,k] @ sW_in, packed like
    # the conv layout: dst[p, ((oc*4+k)*8+ic)*128+c] = N_k[oc*128+c, ic*128+p]
    sW_out64 = sW_out.astype(np.float64)
    sW_in64 = sW_in.astype(np.float64)
    Nk = np.stack([(sW_out64 @ sW_conv[:, :, k].astype(np.float64)
                    @ sW_in64).T for k in range(KC_)], axis=0)  # [k, i, o]
    A = Nk.astype(BF)
    w["w_conv"] = np.ascontiguousarray(
        A.reshape(4, 8, 128, 8, 128).transpose(2, 3, 0, 1, 4)
        .reshape(128, 8 * 32 * 128))
    # collapsed branch biases: row0 dense, row1 full ssm bias, rows2-9 moe
    b_ssm = (sW_out64 @ (sW_conv.astype(np.float64).sum(axis=2)
                         @ sb_in.astype(np.float64)
                         + sb_conv.astype(np.float64))
             + sb_out.astype(np.float64)).astype(f32)
    b10 = np.stack([d2b, b_ssm] + [eW2b for eW2b in eb2], axis=0)
    w["b10"] = b10.astype(BF)                                  # [10, 1024]
    # experts fc1 (fp8 x WSCALE): block idx e*64 + (j*2+ab)*8 + kc
    morder = [ab * 4 + j for j in range(4) for ab in range(2)]
    slabs = []
    for e in range(E):
        Te = _q8(eW1[e].T, WSCALE).reshape(8, 128, 8, 128)    # kc,p,mc,c
        Te = Te[:, :, morder, :].transpose(1, 2, 0, 3)        # p,jm,kc,c
        slabs.append(Te.reshape(128, 64 * 128))
    w["w_e1"] = np.ascontiguousarray(np.concatenate(slabs, axis=1))
    eb1a = np.stack([eb1[e, j * 128:(j + 1) * 128]
                     for e in range(E) for j in range(4)], axis=1)
    eb1b = np.stack([eb1[e, 512 + j * 128: 512 + (j + 1) * 128]
                     for e in range(E) for j in range(4)], axis=1)
    w["b_e1a"] = eb1a.astype(f32)
    w["b_e1b"] = (eb1b * WSCALE).astype(f32)
    # e2 (fp8 x WSCALE): col block ((egrp*8+mc)*16 + el*4 + kc), e=egrp*4+el
    T5 = np.stack([_q8(eW2[e].T, WSCALE).reshape(4, 128, 8, 128)
                   for e in range(E)])                        # e,kc,p,mc,c
    T6 = T5.reshape(2, 4, 4, 128, 8, 128)                     # g,el,kc,p,mc,c
    w["w_e2"] = np.ascontiguousarray(
        T6.transpose(3, 0, 4, 1, 2, 5).reshape(128, E * 32 * 128))
    w["w_d1a"] = _pack_mk(d1W[:HD].T.astype(BF), 8, 32)
    w["w_d1b"] = _pack_mk(d1W[HD:].T.astype(BF), 8, 32)
    w["b_d1a"] = _bias_cols(d1b[:HD], 32)
    w["b_d1b"] = _bias_cols(d1b[HD:], 32)
    # d2: block idx = h*128 + mc*16 + kcl, kg = h*16+kcl
    T4 = d2W.T.astype(BF).reshape(2, 16, 128, 8, 128)         # h,kcl,p,mc,c
    w["w_d2"] = np.ascontiguousarray(
        T4.transpose(2, 0, 3, 1, 4).reshape(128, 256 * 128))
    return w


def make_in_maps(x, weights, ntok=TOK, ncores=NCORE):
    """x [B,T,D] fp32 -> list of per-core in_maps."""
    xt = np.asarray(x, np.float32).reshape(-1, D).T           # [D, tokens]
    in_maps = []
    for c in range(ncores):
        lo = c * ntok
        xc = xt[:, lo:lo + ntok]
        halo = np.zeros((D, HALO), np.float32)
        if lo >= HALO and lo % T != 0:   # conv is causal per batch element
            halo = xt[:, lo - HALO:lo]
        xch = np.concatenate([halo, xc], axis=1)              # [D, nt]
        m = dict(weights)
        xh = xc.astype(BF)
        m["xl_s"] = _featmajor((xc - xh.astype(np.float32)).astype(BF), ntok)
        m["x_s"] = _featmajor(xch.astype(BF), ntok + HALO)
        m["x8_s"] = _featmajor(np.clip(xc, -240.0, 240.0), ntok).astype(E4)
        in_maps.append(m)
    return in_maps


def assemble_output(results, ntok=TOK, ncores=NCORE):
    cols = []
    for c in range(ncores):
        o = results[c]["outT"]                                # [128, 8*ntok]
        cols.append(o.reshape(128, DC, ntok).transpose(1, 0, 2)
                    .reshape(D, ntok))
    full = np.concatenate(cols, axis=1)                       # [D, tokens]
    return np.ascontiguousarray(full.T).reshape(B, T, D).astype(np.float32)


_CACHED = {}


def kernel(**inputs):
    x = np.asarray(inputs["x"], np.float32)
    names = ["rW", "rb", "d1W", "d1b", "d2W", "d2b", "sW_in", "sb_in",
             "sW_conv", "sb_conv", "sW_out", "sb_out", "mW", "mb",
             "eW1", "eb1", "eW2", "eb2"]
    wargs = [np.asarray(inputs[n], np.float32) for n in names]
    if "nc" not in _CACHED:
        _CACHED["nc"] = build_program(TOK)
    nc = _CACHED["nc"]
    weights = pack_weights(*wargs)
    in_maps = make_in_maps(x, weights)
    res = bass_utils.run_bass_kernel_spmd(
        nc, in_maps, core_ids=list(range(NCORE)))
    return assemble_output(res.results)
